# revision 28
# baseline (speedup 1.0000x reference)
"""Trainium2 Bass kernel for a 2-branch GCN (gcn_conv -> leaky_relu -> mean-pool -> fc
head), data-parallel over graphs across 8 NeuronCores.

Math: gcn_conv(x, A, W) = (A_norm @ (x @ W)) + b = ((A_norm @ x) @ W) + b, so we
aggregate raw node features first (gather straight from the input in bf16), then do
the dense 1024x1024 matmul on each core's node shard only. The normalization
dinv[src]*dinv[dst] factorizes: scale x rows by dinv on the host, scale aggregated
rows by dinv on the device; self-loop (1/deg) folds in as a plain self-edge.

v2 layout: graphs are assigned to cores by a joint (branch1, branch2) node-count
balance so both branches fit nwin=10 windows of 128 dst nodes. Within a core, dst
nodes are bin-packed into windows by a greedy that minimizes the window's unique
gathered-row count (a src row used by several dsts of the same window is gathered
once; its one-hot column carries multiple ones via a second "dup" dl layer). Pad
slots use negative gather indices (the DMA skips them); the first three windows
of branch 0 are fully padded with row 0 so stale SBUF is never NaN.

Edges are routed by dst; segment-sum over edges is done on the TensorEngine as
one-hot matmuls (onehot[slot, dst_pos]^T @ gathered_rows accumulated in PSUM).
Pooling contracts nodes on the PE too, producing pool already transposed
[feat, graph] so the FC head needs no further transposes. Output rows per core
are its GPC assigned graphs; the host scatters them back to graph order.
"""

import os
import sys

for _p in ("/opt/trn_rl_repo",):
    if _p not in sys.path:
        sys.path.insert(0, _p)

import numpy as np
import ml_dtypes

import concourse.bacc as bacc
import concourse.mybir as mybir
import concourse.tile as tile
from concourse import bass_utils
from concourse.masks import make_identity

N, E, B, D, OUT_D = 10000, 160000, 64, 1024, 128
NCORES = 8
GPC = B // NCORES  # graphs per core
P = 128
NJ = D // P  # feature chunks (8)

BF16 = ml_dtypes.bfloat16

TRACE = [False]  # test.py can flip this for profiled runs
LAST_RESULTS = [None]

_IOTAM = np.ascontiguousarray(
    np.tile(np.arange(128, dtype=np.float32)[None, :], (128, 1))).astype(BF16)


def _cdiv(a, b):
    return -(-a // b)


def _assign_graphs(batch1, batch2):
    """Joint LPT assignment of the 64 graphs to 8 cores (GPC graphs each),
    balancing node counts of both branches simultaneously."""
    s1 = np.bincount(np.asarray(batch1, np.int64), minlength=B)
    s2 = np.bincount(np.asarray(batch2, np.int64), minlength=B)
    order = np.argsort(-(s1 + s2), kind="stable")
    loads1 = np.zeros(NCORES)
    loads2 = np.zeros(NCORES)
    counts = np.zeros(NCORES, np.int64)
    assign = [[] for _ in range(NCORES)]
    for g in order:
        best = None
        for c in range(NCORES):
            if counts[c] >= GPC:
                continue
            key = (max(loads1[c] + s1[g], loads2[c] + s2[g]),
                   loads1[c] + loads2[c])
            if best is None or key < best[0]:
                best = (key, c)
        c = best[1]
        assign[c].append(int(g))
        loads1[c] += s1[g]
        loads2[c] += s2[g]
        counts[c] += 1
    return assign


def _prep_branch(x, edge_index, batch, assign, fullpad_windows=()):
    """Host-side index preprocessing + array layout for one GCN branch.

    Returns (meta, percore). meta = dict(nwin, cpw, off, totch, seqc, hoff,
    hrows, nrows); percore[c] = dict of arrays named without branch suffix.

    Each window's slots are [home rows | random rows | pads]. Home rows are
    rows whose first use on this core is this window; they are stored
    contiguously (slot order) in the per-core xh array and loaded with one
    sequential DMA. Remaining rows are fetched by indexed gather from xs.
    A slot may serve up to two dst positions (dl + d2 one-hot layers); rows
    with more uses occupy several slots.
    """
    x = np.asarray(x, np.float32)
    src = np.asarray(edge_index[0], np.int64)
    dst = np.asarray(edge_index[1], np.int64)
    batch = np.asarray(batch, np.int64)

    cnt = np.bincount(batch, minlength=B)
    gstart = np.zeros(B + 1, np.int64)
    gstart[1:] = np.cumsum(cnt)

    deg = np.bincount(dst, minlength=N).astype(np.float32) + 1.0
    dinv = (1.0 / np.sqrt(deg)).astype(np.float32)
    xs = (x * dinv[:, None]).astype(BF16)

    corec = np.empty(N, np.int64)
    for c, gs in enumerate(assign):
        for g in gs:
            corec[gstart[g]:gstart[g + 1]] = c

    nodes_c = [int(sum(cnt[g] for g in assign[c])) for c in range(NCORES)]
    nwin = _cdiv(max(nodes_c), P)

    # --- per-core window packing ---
    # percore_win[c] = list over windows of (dsts, slots) where slots is a list
    # of (src_row, pos1, pos2) with pos2 == -1 for single-dst slots.
    order_e = np.argsort(dst, kind="stable")
    es_all, ed_all = src[order_e], dst[order_e]
    dsplit = np.searchsorted(ed_all, np.arange(N + 1))

    percore_win = []
    counts_w = np.zeros((NCORES, nwin), np.int64)
    dups_w = np.zeros((NCORES, nwin), np.int64)
    for c in range(NCORES):
        nodes = np.concatenate(
            [np.arange(gstart[g], gstart[g + 1]) for g in assign[c]])
        # unique-src sets per dst (self included)
        items = []
        for d in nodes:
            s = es_all[dsplit[d]:dsplit[d + 1]]
            u = set(s.tolist())
            u.add(int(d))
            items.append((len(u), int(d), u, s))
        items.sort(key=lambda t: -t[0])
        rows_w = [0] * nwin
        ndst_w = [0] * nwin
        sets = [set() for _ in range(nwin)]
        wins = [[] for _ in range(nwin)]
        for ln, d, u, s in items:
            best = None
            for w in range(nwin):
                if ndst_w[w] >= P:
                    continue
                inc = len(u - sets[w])
                key = (rows_w[w] + inc, ndst_w[w])
                if best is None or key < best[0]:
                    best = (key, w, inc)
            _, w, inc = best
            sets[w] |= u
            rows_w[w] += inc
            ndst_w[w] += 1
            wins[w].append((d, s))
        # slots per window: home rows first (stable storage order), then the
        # rest. A row may be homed in up to HOMES windows (first uses); its
        # remaining uses go through the indexed gather.
        HOMES = int(os.environ.get("K_HOMES", "2"))
        win_slots = []
        homed = {}
        for w in range(nwin):
            # row -> list of dst positions (with edge multiplicity)
            rowmap = {}
            for pos, (d, s) in enumerate(wins[w]):
                for sv in s.tolist():
                    rowmap.setdefault(sv, []).append(pos)
                rowmap.setdefault(int(d), []).append(pos)
            home_slots = []
            rand_slots = []
            for r, plist in rowmap.items():
                slots_r = []
                i = 0
                while i + 1 < len(plist):
                    slots_r.append((r, plist[i], plist[i + 1]))
                    i += 2
                if i < len(plist):
                    slots_r.append((r, plist[i], -1))
                if homed.get(r, 0) < HOMES:
                    homed[r] = homed.get(r, 0) + 1
                    home_slots.append(slots_r[0])
                    rand_slots.extend(slots_r[1:])
                else:
                    rand_slots.extend(slots_r)
            # pairs first within each region -> compact dl2 (dup) chunks
            home_slots.sort(key=lambda t: t[2] < 0)
            rand_slots.sort(key=lambda t: t[2] < 0)
            counts_w[c, w] = len(home_slots) * (1 << 20) + len(rand_slots)
            win_slots.append((wins[w], home_slots, rand_slots))
        percore_win.append(win_slots)

    home_w = counts_w >> 20
    rand_w = counts_w & ((1 << 20) - 1)
    seqc = (home_w.min(axis=0) // P).astype(np.int64)
    # rows demoted from home to random when a core has more homes than seqc*128
    rand_eff = rand_w + (home_w - seqc[None, :] * P)
    randc = _cdiv(rand_eff.max(axis=0), P).astype(np.int64)
    cpw = (seqc + np.maximum(randc, 0)).astype(np.int64)
    off = np.zeros(nwin + 1, np.int64)
    off[1:] = np.cumsum(cpw)
    totch = int(off[-1])
    hoff = np.zeros(nwin + 1, np.int64)
    hoff[1:] = np.cumsum(seqc * P)
    hrows = int(hoff[-1])
    nrows = [int(_cdiv(rand_eff[:, w].max(), 16) * 16) for w in range(nwin)]
    for w in fullpad_windows:
        nrows[w] = int((cpw[w] - seqc[w]) * P)
    # dup (dl2) chunk spans: home pairs live at window chunks [0, hdupc);
    # random pairs at [seqc, seqc + rdupc). Demotion only moves singles, and
    # only when a core has more TOTAL homes than seqc*128; its pairs stay in
    # the home region iff pairs <= seqc*128 — else some pair slots land in the
    # random region start, still inside [seqc, seqc+rdupc) after its singles
    # sort. Track pair counts per region exactly below instead of guessing.
    hpair_w = np.zeros((NCORES, nwin), np.int64)
    rpair_w = np.zeros((NCORES, nwin), np.int64)
    for c in range(NCORES):
        for w in range(nwin):
            _, home_slots, rand_slots = percore_win[c][w]
            nseq = int(seqc[w]) * P
            kept = home_slots[:nseq]
            demoted = home_slots[nseq:]
            hpair_w[c, w] = sum(1 for t in kept if t[2] >= 0)
            merged = [t for t in demoted if t[2] >= 0] + \
                     [t for t in rand_slots if t[2] >= 0]
            rpair_w[c, w] = len(merged)
    hdupc = _cdiv(hpair_w.max(axis=0), P).astype(np.int64)
    rdupc = _cdiv(rpair_w.max(axis=0), P).astype(np.int64)

    percore = []
    for c in range(NCORES):
        g2local = {g: j for j, g in enumerate(assign[c])}
        dl1 = np.full((totch * P,), -1.0, np.float32)
        dl2 = np.full((totch * P,), -1.0, np.float32)
        srcv = np.full((totch * P,), -1, np.int64)
        hcol = [int(hoff[w] // P) * D for w in range(nwin)]
        xh = np.zeros((P, max(hrows // P, 1) * D), BF16)
        dinv_col = np.zeros((P, nwin), np.float32)
        pm4 = np.zeros((P, nwin, GPC), np.float32)
        for w in range(nwin):
            dsts, home_slots, rand_slots = percore_win[c][w]
            nseq = int(seqc[w]) * P
            # demote excess home slots to the random region (pairs first there
            # too, so the dl2 pass stays compact)
            kept = home_slots[:nseq]
            rest = home_slots[nseq:] + rand_slots
            rest.sort(key=lambda t: t[2] < 0)
            slots = kept + rest
            base = off[w] * P
            for i, (r, p1, p2) in enumerate(slots):
                dl1[base + i] = p1
                dl2[base + i] = p2 if p2 >= 0 else -1.0
                if i < nseq:
                    # partition-major home storage: slot i -> row i % P,
                    # column block hcol[w] + (i // P)
                    xh[i % P, hcol[w] + (i // P) * D: hcol[w] + (i // P + 1) * D] \
                        = xs[r]
                else:
                    srcv[base + i] = r
            # pad slots of fully-padded windows gather row 0 (content unused)
            if w in fullpad_windows:
                n = len(slots)
                srcv[base + n: base + cpw[w] * P] = 0
            for pos, (d, s) in enumerate(dsts):
                dinv_col[pos, w] = dinv[d]
                pm4[pos, w, g2local[batch[d]]] = 1.0

        dl1m = np.ascontiguousarray(
            dl1.reshape(totch, P).T).astype(BF16)  # [128, totch]
        dl2m = np.ascontiguousarray(
            dl2.reshape(totch, P).T).astype(BF16)  # [128, totch]

        # int16 gather indices for the random region, wrapped per call:
        # within a call, row e -> [e % 16, base + e // 16], replicated to 128.
        idx = np.full((16, totch * 8), -1, np.int16)
        for w in range(nwin):
            rbase = off[w] * P + int(seqc[w]) * P
            n = nrows[w]
            n1 = min(n, 1024)
            n2 = n - n1
            sw = srcv[rbase: rbase + n1]
            idx[:, off[w] * 8: off[w] * 8 + n1 // 16] = (
                sw.astype(np.int16).reshape(-1, 16).T)
            if n2 > 0:
                sw = srcv[rbase + 1024: rbase + 1024 + n2]
                idx[:, off[w] * 8 + 64: off[w] * 8 + 64 + n2 // 16] = (
                    sw.astype(np.int16).reshape(-1, 16).T)
        idx = np.ascontiguousarray(np.tile(idx, (8, 1)))

        civ = (1.0 / np.maximum(
            [cnt[g] for g in assign[c]], 1)).astype(np.float32)
        ci = np.ascontiguousarray(np.tile(civ, (P, NJ)))  # [128, NJ*GPC]

        percore.append(
            dict(xs=xs, xh=np.ascontiguousarray(xh), dl=dl1m, d2=dl2m, idx=idx,
                 dinv=dinv_col, pm=pm4, ci=ci))

    # pm flatten: [128, nwin*GPC]
    for pc in percore:
        pc["pm"] = np.ascontiguousarray(
            pc["pm"].reshape(P, nwin * GPC).astype(BF16))

    meta = dict(nwin=int(nwin), cpw=tuple(int(v) for v in cpw),
                off=tuple(int(v) for v in off), totch=totch,
                seqc=tuple(int(v) for v in seqc),
                hoff=tuple(int(v) for v in hoff),
                hcols=max(hrows // P, 1) * D,
                hdupc=tuple(int(v) for v in hdupc),
                rdupc=tuple(int(v) for v in rdupc),
                nrows=tuple(int(v) for v in nrows))
    return meta, percore


def _reshape_w(W):
    # [D, D] -> [128, NJ*D]  with [p, j*D + o] = W[j*128 + p, o]
    W = np.asarray(W, np.float32)
    return np.ascontiguousarray(
        W.reshape(NJ, P, D).transpose(1, 0, 2).reshape(P, NJ * D).astype(BF16)
    )


def _reshape_fc(Wfc):
    # [D, OUT_D] -> [128, NJ*OUT_D] fp32
    Wfc = np.asarray(Wfc, np.float32)
    return np.ascontiguousarray(
        Wfc.reshape(NJ, P, OUT_D).transpose(1, 0, 2).reshape(P, NJ * OUT_D)
    )


_PROGRAM_CACHE = {}


def _build_program(meta0, meta1, has_bias, has_fcb, has_finb):
    stage = int(os.environ.get("K_STAGE", "0")) or 99
    f32 = mybir.dt.float32
    bf16 = mybir.dt.bfloat16
    Alu = mybir.AluOpType

    metas = (meta0, meta1)
    nc = bacc.Bacc("TRN2", num_devices=NCORES, debug=False)

    cwmax = max(max(meta0["cpw"]), max(meta1["cpw"]))

    # DRAM tensors
    xs_d, xh_d, dl_d, d2_d, idx_d, dinv_d, pm_d, ci_d, W_d, bias_d, fc_d, fcb_d = (
        [], [], [], [], [], [], [], [], [], [], [], [])
    for b in (0, 1):
        m = metas[b]
        xs_d.append(nc.dram_tensor(f"xs{b}", [N, D], bf16, kind="ExternalInput"))
        xh_d.append(nc.dram_tensor(f"xh{b}", [P, m["hcols"]], bf16,
                                   kind="ExternalInput"))
        dl_d.append(nc.dram_tensor(f"dl{b}", [P, m["totch"]], bf16,
                                   kind="ExternalInput"))
        d2_d.append(nc.dram_tensor(f"d2{b}", [P, m["totch"]], bf16,
                                   kind="ExternalInput"))
        idx_d.append(nc.dram_tensor(f"idx{b}", [P, m["totch"] * 8], mybir.dt.int16,
                                    kind="ExternalInput"))
        dinv_d.append(nc.dram_tensor(f"dinv{b}", [P, m["nwin"]], f32,
                                     kind="ExternalInput"))
        pm_d.append(nc.dram_tensor(f"pm{b}", [P, m["nwin"] * GPC], bf16,
                                   kind="ExternalInput"))
        ci_d.append(nc.dram_tensor(f"ci{b}", [P, NJ * GPC], f32,
                                   kind="ExternalInput"))
        W_d.append(nc.dram_tensor(f"W{b}", [P, NJ * D], bf16, kind="ExternalInput"))
        fc_d.append(nc.dram_tensor(f"fc{b}", [P, NJ * OUT_D], f32,
                                   kind="ExternalInput"))
        if has_bias[b]:
            bias_d.append(nc.dram_tensor(f"bias{b}", [1, D], bf16,
                                         kind="ExternalInput"))
        else:
            bias_d.append(None)
        if has_fcb[b]:
            fcb_d.append(nc.dram_tensor(f"fcb{b}", [1, OUT_D], f32,
                                        kind="ExternalInput"))
        else:
            fcb_d.append(None)
    fin_d = nc.dram_tensor("fin", [P, 2], f32, kind="ExternalInput")
    iota_d = nc.dram_tensor("iotam", [P, P], bf16, kind="ExternalInput")
    finb_d = nc.dram_tensor("finb", [1, 1], f32, kind="ExternalInput") if has_finb else None
    out_d = nc.dram_tensor("out", [GPC, 1], f32, kind="ExternalOutput")

    with tile.TileContext(nc) as tc:
        with (
            tc.tile_pool(name="const", bufs=1) as cpool,
            tc.tile_pool(name="xg", bufs=3) as xgpool,
            tc.tile_pool(name="ohp", bufs=2) as ohpool,
            tc.tile_pool(name="work", bufs=2) as wpool,
            tc.tile_pool(name="pz", bufs=1, space="PSUM") as pz,
            tc.tile_pool(name="pzt", bufs=1, space="PSUM") as pzt,
            tc.tile_pool(name="ph", bufs=1, space="PSUM") as ph,
            tc.tile_pool(name="pacc", bufs=1, space="PSUM") as pacc,
        ):
            identity = cpool.tile([P, P], f32, tag="ident", name="ident")
            make_identity(nc, identity[:])
            identity_bf = cpool.tile([P, P], bf16, tag="identbf", name="identbf")
            make_identity(nc, identity_bf[:])
            iotam = cpool.tile([P, P], bf16, tag="iotam", name="iotam")
            nc.sync.dma_start(out=iotam[:], in_=iota_d.ap())

            # resident small/medium tensors
            W_sb, fc_sb, dinv_sb, pm_sb, ci_sb, idx_sb, bias_sb, fcb_sb = (
                [], [], [], [], [], [], [], [])
            dl_sb, d2_sb = [], []
            poolacc_sb = []
            poolT_sb = []
            now = os.environ.get("K_NOW", "") == "1"
            for b in (0, 1):
                m = metas[b]
                t = cpool.tile([P, m["totch"] * 8], mybir.dt.int16, tag=f"idx{b}",
                               name=f"idx{b}sb")
                nc.sync.dma_start(out=t[:], in_=idx_d[b].ap())
                idx_sb.append(t)
                t = cpool.tile([P, m["totch"]], bf16, tag=f"dl{b}", name=f"dl{b}sb")
                nc.sync.dma_start(out=t[:], in_=dl_d[b].ap())
                dl_sb.append(t)
                t = cpool.tile([P, m["totch"]], bf16, tag=f"d2{b}", name=f"d2{b}sb")
                nc.sync.dma_start(out=t[:], in_=d2_d[b].ap())
                d2_sb.append(t)
                t = cpool.tile([P, m["nwin"]], f32, tag=f"dinv{b}", name=f"dinv{b}sb")
                nc.sync.dma_start(out=t[:], in_=dinv_d[b].ap())
                dinv_sb.append(t)
                t = cpool.tile([P, m["nwin"] * GPC], bf16, tag=f"pm{b}", name=f"pm{b}sb")
                nc.sync.dma_start(out=t[:], in_=pm_d[b].ap())
                pm_sb.append(t)
                t = cpool.tile([P, NJ * GPC], f32, tag=f"ci{b}", name=f"ci{b}sb")
                nc.sync.dma_start(out=t[:], in_=ci_d[b].ap())
                ci_sb.append(t)
                if has_bias[b]:
                    t = cpool.tile([1, D], bf16, tag=f"bias{b}", name=f"bias{b}sb")
                    nc.sync.dma_start(out=t[:], in_=bias_d[b].ap())
                    bias_sb.append(t)
                else:
                    bias_sb.append(None)
                if has_fcb[b]:
                    t = cpool.tile([1, OUT_D], f32, tag=f"fcb{b}", name=f"fcb{b}sb")
                    nc.sync.dma_start(out=t[:], in_=fcb_d[b].ap())
                    fcb_sb.append(t)
                else:
                    fcb_sb.append(None)
                poolT_sb.append(
                    cpool.tile([P, NJ * GPC], f32, tag=f"pT{b}", name=f"pT{b}sb"))
                poolacc_sb.append(
                    cpool.tile([GPC, D], f32, tag=f"pa{b}", name=f"pa{b}sb"))
            for b in (0, 1):
                t = cpool.tile([P, NJ * D], bf16, tag=f"W{b}", name=f"W{b}sb")
                if not now:
                    nc.sync.dma_start(out=t[:], in_=W_d[b].ap())
                W_sb.append(t)
                t = cpool.tile([P, NJ * OUT_D], f32, tag=f"fc{b}", name=f"fc{b}sb")
                if not now:
                    nc.sync.dma_start(out=t[:], in_=fc_d[b].ap())
                fc_sb.append(t)
            fin_sb = cpool.tile([P, 2], f32, tag="fin", name="finsb")
            nc.sync.dma_start(out=fin_sb[:], in_=fin_d.ap())
            if has_finb:
                finb_sb = cpool.tile([1, 1], f32, tag="finb", name="finbsb")
                nc.sync.dma_start(out=finb_sb[:], in_=finb_d.ap())
            if has_bias[0] or has_bias[1]:
                ones_bf = cpool.tile([1, P], bf16, tag="ones_bf", name="ones_bf")
                nc.vector.memset(ones_bf[:], 1.0)
            if has_fcb[0] or has_fcb[1] or has_finb:
                ones8 = cpool.tile([1, GPC], f32, tag="ones8", name="ones8")
                nc.vector.memset(ones8[:], 1.0)

            def do_branch(b):
                m = metas[b]
                nwin, cpw, off = m["nwin"], m["cpw"], m["off"]
                seqc, hoff, nrows = m["seqc"], m["hoff"], m["nrows"]
                hdupc, rdupc = m["hdupc"], m["rdupc"]
                maxwin = int(os.environ.get("K_MAXWIN", "0")) or nwin
                nwin_eff = min(nwin, maxwin)
                pool_ps = pacc.tile([GPC, D], f32, tag="pacc", name=f"pacc{b}")
                for w in range(nwin_eff):
                    cw, ofs = cpw[w], off[w]
                    sq = seqc[w]
                    xg = xgpool.tile([P, cwmax * D], bf16, tag="xg",
                                     name=f"xg{b}_{w}")
                    if sq > 0:
                        hcol = (hoff[w] // P) * D
                        nc.sync.dma_start(
                            out=xg[:, 0:sq * D],
                            in_=xh_d[b].ap()[:, hcol:hcol + sq * D])
                    n = nrows[w]
                    n1 = min(n, 1024)
                    n2 = n - n1
                    if n1 > 0:
                        nc.gpsimd.dma_gather(
                            out_ap=xg[:, sq * D:(sq + _cdiv(n1, P)) * D].rearrange(
                                "p (c f) -> p c f", f=D),
                            in_ap=xs_d[b].ap(),
                            idxs_ap=idx_sb[b][:, ofs * 8: ofs * 8 + n1 // 16],
                            num_idxs=n1,
                            num_idxs_reg=n1,
                            elem_size=D,
                            single_packet=False,
                        )
                    if n2 > 0:
                        nc.gpsimd.dma_gather(
                            out_ap=xg[:, (sq + 8) * D: (sq + 8 + _cdiv(n2, P)) * D]
                                .rearrange("p (c f) -> p c f", f=D),
                            in_ap=xs_d[b].ap(),
                            idxs_ap=idx_sb[b][:, ofs * 8 + 64: ofs * 8 + 64 + n2 // 16],
                            num_idxs=n2,
                            num_idxs_reg=n2,
                            elem_size=D,
                            single_packet=False,
                        )
                    oh = ohpool.tile([P, cwmax * P], bf16, tag="oh",
                                     name=f"oh{b}_{w}")
                    nc.vector.tensor_tensor(
                        out=oh[:, 0:cw * P].rearrange("p (c d) -> p c d", d=P),
                        in0=dl_sb[b][:, ofs:ofs + cw].to_broadcast([P, cw, P]),
                        in1=iotam[:].rearrange("p (c d) -> p c d", c=1)
                            .to_broadcast([P, cw, P]),
                        op=Alu.is_equal)
                    dup_spans = []
                    if hdupc[w] > 0:
                        dup_spans.append((0, hdupc[w]))
                    if rdupc[w] > 0:
                        dup_spans.append((sq, sq + rdupc[w]))
                    for si, (c0, c1) in enumerate(dup_spans):
                        dc = c1 - c0
                        oh2 = ohpool.tile([P, cwmax * P], bf16, tag="oh2",
                                          name=f"oh2{b}_{w}_{si}")
                        nc.vector.tensor_tensor(
                            out=oh2[:, 0:dc * P].rearrange("p (c d) -> p c d", d=P),
                            in0=d2_sb[b][:, ofs + c0:ofs + c1]
                                .to_broadcast([P, dc, P]),
                            in1=iotam[:].rearrange("p (c d) -> p c d", c=1)
                                .to_broadcast([P, dc, P]),
                            op=Alu.is_equal)
                        nc.vector.tensor_tensor(
                            out=oh[:, c0 * P:c1 * P], in0=oh[:, c0 * P:c1 * P],
                            in1=oh2[:, 0:dc * P], op=Alu.add)

                    z_ps = pz.tile([P, D], f32, tag="z", name=f"z{b}_{w}")
                    for c in range(cw):
                        lhsT = oh[:, c * P:(c + 1) * P]
                        st = c == 0
                        sp = c == cw - 1
                        nc.tensor.matmul(z_ps[:, 0:512], lhsT,
                                         xg[:, c * D:c * D + 512],
                                         start=st, stop=sp)
                        nc.tensor.matmul(z_ps[:, 512:1024], lhsT,
                                         xg[:, c * D + 512:(c + 1) * D],
                                         start=st, stop=sp)

                    z_sb = wpool.tile([P, D], bf16, tag="z_sb", name=f"zsb{b}_{w}")
                    nc.scalar.activation(
                        out=z_sb[:, 0:512], in_=z_ps[:, 0:512],
                        func=mybir.ActivationFunctionType.Copy,
                        scale=dinv_sb[b][:, w:w + 1])
                    nc.vector.tensor_scalar(
                        out=z_sb[:, 512:1024], in0=z_ps[:, 512:1024],
                        scalar1=dinv_sb[b][:, w:w + 1], scalar2=None,
                        op0=Alu.mult)
                    if stage < 2:
                        nc.vector.tensor_tensor(out=poolT_sb[b][:, 0:1],
                                                in0=poolT_sb[b][:, 0:1],
                                                in1=z_sb[:, 0:1], op=Alu.add)
                        continue

                    zT_ps = pzt.tile([P, D], bf16, tag="zt", name=f"zt{b}_{w}")
                    for j in range(NJ):
                        nc.tensor.transpose(
                            zT_ps[:, j * P:(j + 1) * P],
                            z_sb[:, j * P:(j + 1) * P],
                            identity_bf[:])
                    zT_sb = wpool.tile([P, D], bf16, tag="zt_sb", name=f"ztsb{b}_{w}")
                    nc.scalar.copy(out=zT_sb[:, 0:512], in_=zT_ps[:, 0:512])
                    nc.vector.tensor_copy(out=zT_sb[:, 512:1024],
                                          in_=zT_ps[:, 512:1024])
                    if stage < 3:
                        nc.vector.tensor_tensor(out=poolT_sb[b][:, 0:1],
                                                in0=poolT_sb[b][:, 0:1],
                                                in1=zT_sb[:, 0:1], op=Alu.add)
                        continue

                    h_ps = ph.tile([P, D], f32, tag="h", name=f"h{b}_{w}")
                    for j in range(NJ):
                        lhsT = zT_sb[:, j * P:(j + 1) * P]
                        st = j == 0
                        sp = (j == NJ - 1) and not has_bias[b]
                        nc.tensor.matmul(h_ps[:, 0:512], lhsT,
                                         W_sb[b][:, j * D:j * D + 512],
                                         start=st, stop=sp)
                        nc.tensor.matmul(h_ps[:, 512:1024], lhsT,
                                         W_sb[b][:, j * D + 512:(j + 1) * D],
                                         start=st, stop=sp)
                    if has_bias[b]:
                        nc.tensor.matmul(h_ps[:, 0:512], ones_bf[:],
                                         bias_sb[b][:, 0:512], start=False, stop=True)
                        nc.tensor.matmul(h_ps[:, 512:1024], ones_bf[:],
                                         bias_sb[b][:, 512:1024], start=False,
                                         stop=True)

                    y = wpool.tile([P, D], bf16, tag="y", name=f"y{b}_{w}")
                    nc.scalar.activation(
                        out=y[:], in_=h_ps[:],
                        func=mybir.ActivationFunctionType.Lrelu, alpha=0.01)
                    if stage < 4:
                        nc.vector.tensor_tensor(out=poolT_sb[b][:, 0:1],
                                                in0=poolT_sb[b][:, 0:1],
                                                in1=y[:, 0:1], op=Alu.add)
                        continue

                    plhsT = pm_sb[b][:, w * GPC:(w + 1) * GPC]
                    st = w == 0
                    sp = w == nwin_eff - 1
                    nc.tensor.matmul(pool_ps[:, 0:512], plhsT, y[:, 0:512],
                                     start=st, stop=sp, skip_group_check=True)
                    nc.tensor.matmul(pool_ps[:, 512:1024], plhsT,
                                     y[:, 512:1024], start=st, stop=sp,
                                     skip_group_check=True)

                nc.scalar.copy(out=poolacc_sb[b][:], in_=pool_ps[:])
                pt_ps = pzt.tile([P, NJ * GPC], f32, tag="ptt", name=f"pt{b}ps")
                for j in range(NJ):
                    nc.tensor.transpose(
                        pt_ps[:, j * GPC:(j + 1) * GPC],
                        poolacc_sb[b][0:GPC, j * P:(j + 1) * P],
                        identity[0:GPC, 0:GPC])
                nc.vector.tensor_tensor(out=poolT_sb[b][:],
                                        in0=pt_ps[:, 0:NJ * GPC],
                                        in1=ci_sb[b][:], op=Alu.mult)

            def head_branch(b):
                # first FC layer for branch b: y1T[b] = lrelu(fc^T @ poolT)
                h1_full = ph.tile([P, D], f32, tag="h", name=f"h1_{b}ps")
                h1_ps = h1_full[:, 0:GPC]
                for j in range(NJ):
                    nc.tensor.matmul(
                        h1_ps,
                        fc_sb[b][:, j * OUT_D:(j + 1) * OUT_D],
                        poolT_sb[b][:, j * GPC:(j + 1) * GPC],
                        start=(j == 0), stop=(j == NJ - 1) and not has_fcb[b])
                if has_fcb[b]:
                    nc.tensor.matmul(h1_ps, fcb_sb[b][:],
                                     ones8[:], start=False, stop=True)
                t = cpool.tile([P, GPC], f32, tag=f"y1T{b}", name=f"y1T{b}sb")
                nc.scalar.activation(
                    out=t[:], in_=h1_ps,
                    func=mybir.ActivationFunctionType.Lrelu, alpha=0.01)
                return t

            repeat = int(os.environ.get("K_REPEAT", "1"))
            nohead = os.environ.get("K_NOHEAD", "") == "1"
            for _rep in range(repeat):
                y1T = [None, None]
                for b in (0, 1):
                    do_branch(b)
                    if not nohead:
                        y1T[b] = head_branch(b)

                if nohead:
                    out_sb = cpool.tile([GPC, 1], f32, tag="out_sb", name="out_sb")
                    nc.vector.tensor_copy(out=out_sb[:], in_=poolT_sb[0][0:GPC, 0:1])
                    nc.sync.dma_start(out=out_d.ap(), in_=out_sb[:])
                else:
                    out_full = pz.tile([P, D], f32, tag="z", name="outps")
                    out_ps = out_full[0:GPC, 0:1]
                    nc.tensor.matmul(out_ps, y1T[0][:],
                                     fin_sb[:, 0:1], start=True, stop=False)
                    nc.tensor.matmul(out_ps, y1T[1][:],
                                     fin_sb[:, 1:2],
                                     start=False, stop=not has_finb)
                    if has_finb:
                        nc.tensor.matmul(out_ps, ones8[:],
                                         finb_sb[:], start=False, stop=True)
                    out_sb = cpool.tile([GPC, 1], f32, tag="out_sb", name="out_sb")
                    nc.vector.tensor_copy(out=out_sb[:], in_=out_ps)
                    nc.sync.dma_start(out=out_d.ap(), in_=out_sb[:])

    nc.compile()
    return nc


def _prep_all(inputs):
    """Full host prep: graph assignment, both branch metas, per-core input maps."""
    assign = _assign_graphs(inputs["pro1_batch"], inputs["pro2_batch"])
    meta0, pc0 = _prep_branch(inputs["pro1_x"], inputs["pro1_edge_index"],
                              inputs["pro1_batch"], assign,
                              fullpad_windows=(0, 1, 2))
    meta1, pc1 = _prep_branch(inputs["pro2_x"], inputs["pro2_edge_index"],
                              inputs["pro2_batch"], assign)

    Wr = (_reshape_w(inputs["W1"]), _reshape_w(inputs["W2"]))
    fcr = (_reshape_fc(inputs["fc1_W"]), _reshape_fc(inputs["fc2_W"]))
    fin = np.ascontiguousarray(
        np.asarray(inputs["final_W"], np.float32).reshape(2, P).T)

    b1 = np.asarray(inputs["b1"], np.float32)
    b2 = np.asarray(inputs["b2"], np.float32)
    fc1_b = np.asarray(inputs["fc1_b"], np.float32)
    fc2_b = np.asarray(inputs["fc2_b"], np.float32)
    final_b = np.asarray(inputs["final_b"], np.float32)
    has_bias = (bool(np.any(b1)), bool(np.any(b2)))
    has_fcb = (bool(np.any(fc1_b)), bool(np.any(fc2_b)))
    has_finb = bool(np.any(final_b))

    in_maps = []
    for c in range(NCORES):
        m = {}
        for b, pc in ((0, pc0), (1, pc1)):
            d = pc[c]
            m[f"xs{b}"] = d["xs"]
            m[f"xh{b}"] = d["xh"]
            m[f"dl{b}"] = d["dl"]
            m[f"d2{b}"] = d["d2"]
            m[f"idx{b}"] = d["idx"]
            m[f"dinv{b}"] = d["dinv"]
            m[f"pm{b}"] = d["pm"]
            m[f"ci{b}"] = d["ci"]
            m[f"W{b}"] = Wr[b]
            m[f"fc{b}"] = fcr[b]
            if has_bias[b]:
                m[f"bias{b}"] = (b1 if b == 0 else b2).reshape(1, D).astype(BF16)
            if has_fcb[b]:
                m[f"fcb{b}"] = (fc1_b if b == 0 else fc2_b).reshape(1, OUT_D)
        m["fin"] = fin
        m["iotam"] = _IOTAM
        if has_finb:
            m["finb"] = final_b.reshape(1, 1)
        in_maps.append(m)

    return assign, meta0, meta1, has_bias, has_fcb, has_finb, in_maps


def kernel(pro1_x, pro1_edge_index, pro1_batch, pro2_x, pro2_edge_index, pro2_batch,
           W1, b1, fc1_W, fc1_b, W2, b2, fc2_W, fc2_b, final_W, final_b):
    inputs = dict(pro1_x=pro1_x, pro1_edge_index=pro1_edge_index,
                  pro1_batch=pro1_batch, pro2_x=pro2_x,
                  pro2_edge_index=pro2_edge_index, pro2_batch=pro2_batch,
                  W1=W1, b1=b1, fc1_W=fc1_W, fc1_b=fc1_b,
                  W2=W2, b2=b2, fc2_W=fc2_W, fc2_b=fc2_b,
                  final_W=final_W, final_b=final_b)
    (assign, meta0, meta1, has_bias, has_fcb, has_finb,
     in_maps) = _prep_all(inputs)

    key = (meta0["nwin"], meta0["cpw"], meta0["seqc"], meta0["nrows"],
           meta0["hdupc"], meta0["rdupc"],
           meta1["nwin"], meta1["cpw"], meta1["seqc"], meta1["nrows"],
           meta1["hdupc"], meta1["rdupc"],
           has_bias, has_fcb, has_finb)
    nc = _PROGRAM_CACHE.get(key)
    if nc is None:
        nc = _build_program(meta0, meta1, has_bias, has_fcb, has_finb)
        _PROGRAM_CACHE[key] = nc

    res = bass_utils.run_bass_kernel_spmd(
        nc, in_maps, core_ids=list(range(NCORES)), trace=TRACE[0])
    LAST_RESULTS[0] = res
    out = np.zeros((B, 1), np.float32)
    for c in range(NCORES):
        oc = np.asarray(res.results[c]["out"], np.float32)
        for j, g in enumerate(assign[c]):
            out[g, 0] = oc[j, 0]
    return out


# revision 29
# speedup vs baseline: 1.2819x; 1.2819x over previous
"""Trainium2 Bass kernel for a 2-branch GCN (gcn_conv -> leaky_relu -> mean-pool -> fc
head), data-parallel over graphs across 8 NeuronCores.

Math: gcn_conv(x, A, W) = (A_norm @ (x @ W)) + b = ((A_norm @ x) @ W) + b, so we
aggregate raw node features first (gather straight from the input in bf16), then do
the dense 1024x1024 matmul on each core's node shard only. The normalization
dinv[src]*dinv[dst] factorizes: scale x rows by dinv on the host, scale aggregated
rows by dinv on the device; self-loop (1/deg) folds in as a plain self-edge.

v2 layout: graphs are assigned to cores by a joint (branch1, branch2) node-count
balance so both branches fit nwin=10 windows of 128 dst nodes. Within a core, dst
nodes are bin-packed into windows by a greedy that minimizes the window's unique
gathered-row count (a src row used by several dsts of the same window is gathered
once; its one-hot column carries multiple ones via a second "dup" dl layer). Pad
slots use negative gather indices (the DMA skips them); the first three windows
of branch 0 are fully padded with row 0 so stale SBUF is never NaN.

Edges are routed by dst; segment-sum over edges is done on the TensorEngine as
one-hot matmuls (onehot[slot, dst_pos]^T @ gathered_rows accumulated in PSUM).
Pooling contracts nodes on the PE too, producing pool already transposed
[feat, graph] so the FC head needs no further transposes. Output rows per core
are its GPC assigned graphs; the host scatters them back to graph order.
"""

import os
import sys

for _p in ("/opt/trn_rl_repo",):
    if _p not in sys.path:
        sys.path.insert(0, _p)

import numpy as np
import ml_dtypes

import concourse.bacc as bacc
import concourse.mybir as mybir
import concourse.tile as tile
from concourse import bass_utils
from concourse.masks import make_identity

N, E, B, D, OUT_D = 10000, 160000, 64, 1024, 128
NCORES = 8
GPC = B // NCORES  # graphs per core
P = 128
NJ = D // P  # feature chunks (8)

BF16 = ml_dtypes.bfloat16

TRACE = [False]  # test.py can flip this for profiled runs
LAST_RESULTS = [None]

_IOTAM = np.ascontiguousarray(
    np.tile(np.arange(128, dtype=np.float32)[None, :], (128, 1))).astype(BF16)


def _cdiv(a, b):
    return -(-a // b)


def _assign_graphs(batch1, batch2):
    """Joint LPT assignment of the 64 graphs to 8 cores (GPC graphs each),
    balancing node counts of both branches simultaneously."""
    s1 = np.bincount(np.asarray(batch1, np.int64), minlength=B)
    s2 = np.bincount(np.asarray(batch2, np.int64), minlength=B)
    order = np.argsort(-(s1 + s2), kind="stable")
    loads1 = np.zeros(NCORES)
    loads2 = np.zeros(NCORES)
    counts = np.zeros(NCORES, np.int64)
    assign = [[] for _ in range(NCORES)]
    for g in order:
        best = None
        for c in range(NCORES):
            if counts[c] >= GPC:
                continue
            key = (max(loads1[c] + s1[g], loads2[c] + s2[g]),
                   loads1[c] + loads2[c])
            if best is None or key < best[0]:
                best = (key, c)
        c = best[1]
        assign[c].append(int(g))
        loads1[c] += s1[g]
        loads2[c] += s2[g]
        counts[c] += 1
    return assign


def _prep_branch(x, edge_index, batch, assign, fullpad_windows=()):
    """Host-side index preprocessing + array layout for one GCN branch.

    Returns (meta, percore). meta = dict(nwin, cpw, off, totch, seqc, hoff,
    hrows, nrows); percore[c] = dict of arrays named without branch suffix.

    Each window's slots are [home rows | random rows | pads]. Home rows are
    rows whose first use on this core is this window; they are stored
    contiguously (slot order) in the per-core xh array and loaded with one
    sequential DMA. Remaining rows are fetched by indexed gather from xs.
    A slot may serve up to two dst positions (dl + d2 one-hot layers); rows
    with more uses occupy several slots.
    """
    x = np.asarray(x, np.float32)
    src = np.asarray(edge_index[0], np.int64)
    dst = np.asarray(edge_index[1], np.int64)
    batch = np.asarray(batch, np.int64)

    cnt = np.bincount(batch, minlength=B)
    gstart = np.zeros(B + 1, np.int64)
    gstart[1:] = np.cumsum(cnt)

    deg = np.bincount(dst, minlength=N).astype(np.float32) + 1.0
    dinv = (1.0 / np.sqrt(deg)).astype(np.float32)
    xs = (x * dinv[:, None]).astype(BF16)

    corec = np.empty(N, np.int64)
    for c, gs in enumerate(assign):
        for g in gs:
            corec[gstart[g]:gstart[g + 1]] = c

    nodes_c = [int(sum(cnt[g] for g in assign[c])) for c in range(NCORES)]
    nwin = _cdiv(max(nodes_c), P)

    # --- per-core window packing ---
    # percore_win[c] = list over windows of (dsts, slots) where slots is a list
    # of (src_row, pos1, pos2) with pos2 == -1 for single-dst slots.
    order_e = np.argsort(dst, kind="stable")
    es_all, ed_all = src[order_e], dst[order_e]
    dsplit = np.searchsorted(ed_all, np.arange(N + 1))

    percore_win = []
    counts_w = np.zeros((NCORES, nwin), np.int64)
    dups_w = np.zeros((NCORES, nwin), np.int64)
    for c in range(NCORES):
        nodes = np.concatenate(
            [np.arange(gstart[g], gstart[g + 1]) for g in assign[c]])
        # unique-src sets per dst (self included)
        items = []
        for d in nodes:
            s = es_all[dsplit[d]:dsplit[d + 1]]
            u = set(s.tolist())
            u.add(int(d))
            items.append((len(u), int(d), u, s))
        items.sort(key=lambda t: -t[0])
        rows_w = [0] * nwin
        ndst_w = [0] * nwin
        sets = [set() for _ in range(nwin)]
        wins = [[] for _ in range(nwin)]
        for ln, d, u, s in items:
            best = None
            for w in range(nwin):
                if ndst_w[w] >= P:
                    continue
                inc = len(u - sets[w])
                key = (rows_w[w] + inc, ndst_w[w])
                if best is None or key < best[0]:
                    best = (key, w, inc)
            _, w, inc = best
            sets[w] |= u
            rows_w[w] += inc
            ndst_w[w] += 1
            wins[w].append((d, s))
        # slots per window: home rows first (stable storage order), then the
        # rest. A row may be homed in up to HOMES windows (first uses); its
        # remaining uses go through the indexed gather.
        HOMES = int(os.environ.get("K_HOMES", "2"))
        win_slots = []
        homed = {}
        for w in range(nwin):
            # row -> list of dst positions (with edge multiplicity)
            rowmap = {}
            for pos, (d, s) in enumerate(wins[w]):
                for sv in s.tolist():
                    rowmap.setdefault(sv, []).append(pos)
                rowmap.setdefault(int(d), []).append(pos)
            home_slots = []
            rand_slots = []
            for r, plist in rowmap.items():
                slots_r = []
                i = 0
                while i + 1 < len(plist):
                    slots_r.append((r, plist[i], plist[i + 1]))
                    i += 2
                if i < len(plist):
                    slots_r.append((r, plist[i], -1))
                if homed.get(r, 0) < HOMES:
                    homed[r] = homed.get(r, 0) + 1
                    home_slots.append(slots_r[0])
                    rand_slots.extend(slots_r[1:])
                else:
                    rand_slots.extend(slots_r)
            # pairs first within each region -> compact dl2 (dup) chunks
            home_slots.sort(key=lambda t: t[2] < 0)
            rand_slots.sort(key=lambda t: t[2] < 0)
            counts_w[c, w] = len(home_slots) * (1 << 20) + len(rand_slots)
            win_slots.append((wins[w], home_slots, rand_slots))
        percore_win.append(win_slots)

    home_w = counts_w >> 20
    rand_w = counts_w & ((1 << 20) - 1)
    seqc = (home_w.min(axis=0) // P).astype(np.int64)
    # rows demoted from home to random when a core has more homes than seqc*128
    rand_eff = rand_w + (home_w - seqc[None, :] * P)
    randc = _cdiv(rand_eff.max(axis=0), P).astype(np.int64)
    cpw = (seqc + np.maximum(randc, 0)).astype(np.int64)
    off = np.zeros(nwin + 1, np.int64)
    off[1:] = np.cumsum(cpw)
    totch = int(off[-1])
    hoff = np.zeros(nwin + 1, np.int64)
    hoff[1:] = np.cumsum(seqc * P)
    hrows = int(hoff[-1])
    nrows = [int(_cdiv(rand_eff[:, w].max(), 16) * 16) for w in range(nwin)]
    for w in fullpad_windows:
        nrows[w] = int((cpw[w] - seqc[w]) * P)
    # dup (dl2) chunk spans: home pairs live at window chunks [0, hdupc);
    # random pairs at [seqc, seqc + rdupc). Demotion only moves singles, and
    # only when a core has more TOTAL homes than seqc*128; its pairs stay in
    # the home region iff pairs <= seqc*128 — else some pair slots land in the
    # random region start, still inside [seqc, seqc+rdupc) after its singles
    # sort. Track pair counts per region exactly below instead of guessing.
    hpair_w = np.zeros((NCORES, nwin), np.int64)
    rpair_w = np.zeros((NCORES, nwin), np.int64)
    for c in range(NCORES):
        for w in range(nwin):
            _, home_slots, rand_slots = percore_win[c][w]
            nseq = int(seqc[w]) * P
            kept = home_slots[:nseq]
            demoted = home_slots[nseq:]
            hpair_w[c, w] = sum(1 for t in kept if t[2] >= 0)
            merged = [t for t in demoted if t[2] >= 0] + \
                     [t for t in rand_slots if t[2] >= 0]
            rpair_w[c, w] = len(merged)
    hdupc = _cdiv(hpair_w.max(axis=0), P).astype(np.int64)
    rdupc = _cdiv(rpair_w.max(axis=0), P).astype(np.int64)

    percore = []
    for c in range(NCORES):
        g2local = {g: j for j, g in enumerate(assign[c])}
        dl1 = np.full((totch * P,), -1.0, np.float32)
        dl2 = np.full((totch * P,), -1.0, np.float32)
        srcv = np.full((totch * P,), -1, np.int64)
        hcol = [int(hoff[w] // P) * D for w in range(nwin)]
        xh = np.zeros((P, max(hrows // P, 1) * D), BF16)
        dinv_col = np.zeros((P, nwin), np.float32)
        pm4 = np.zeros((P, nwin, GPC), np.float32)
        for w in range(nwin):
            dsts, home_slots, rand_slots = percore_win[c][w]
            nseq = int(seqc[w]) * P
            # demote excess home slots to the random region (pairs first there
            # too, so the dl2 pass stays compact)
            kept = home_slots[:nseq]
            rest = home_slots[nseq:] + rand_slots
            rest.sort(key=lambda t: t[2] < 0)
            slots = kept + rest
            base = off[w] * P
            for i, (r, p1, p2) in enumerate(slots):
                dl1[base + i] = p1
                dl2[base + i] = p2 if p2 >= 0 else -1.0
                if i < nseq:
                    # partition-major home storage: slot i -> row i % P,
                    # column block hcol[w] + (i // P)
                    xh[i % P, hcol[w] + (i // P) * D: hcol[w] + (i // P + 1) * D] \
                        = xs[r]
                else:
                    srcv[base + i] = r
            # pad slots of fully-padded windows gather row 0 (content unused)
            if w in fullpad_windows:
                n = len(slots)
                srcv[base + n: base + cpw[w] * P] = 0
            for pos, (d, s) in enumerate(dsts):
                dinv_col[pos, w] = dinv[d]
                pm4[pos, w, g2local[batch[d]]] = 1.0

        dl1m = np.ascontiguousarray(
            dl1.reshape(totch, P).T).astype(BF16)  # [128, totch]
        dl2m = np.ascontiguousarray(
            dl2.reshape(totch, P).T).astype(BF16)  # [128, totch]

        # int16 gather indices for the random region, wrapped per call:
        # within a call, row e -> [e % 16, base + e // 16], replicated to 128.
        idx = np.full((16, totch * 8), -1, np.int16)
        for w in range(nwin):
            rbase = off[w] * P + int(seqc[w]) * P
            n = nrows[w]
            n1 = min(n, 1024)
            n2 = n - n1
            sw = srcv[rbase: rbase + n1]
            idx[:, off[w] * 8: off[w] * 8 + n1 // 16] = (
                sw.astype(np.int16).reshape(-1, 16).T)
            if n2 > 0:
                sw = srcv[rbase + 1024: rbase + 1024 + n2]
                idx[:, off[w] * 8 + 64: off[w] * 8 + 64 + n2 // 16] = (
                    sw.astype(np.int16).reshape(-1, 16).T)
        idx = np.ascontiguousarray(np.tile(idx, (8, 1)))

        civ = (1.0 / np.maximum(
            [cnt[g] for g in assign[c]], 1)).astype(np.float32)
        ci = np.ascontiguousarray(np.tile(civ, (P, NJ)))  # [128, NJ*GPC]

        percore.append(
            dict(xs=xs, xh=np.ascontiguousarray(xh), dl=dl1m, d2=dl2m, idx=idx,
                 dinv=dinv_col, pm=pm4, ci=ci))

    # pm flatten: [128, nwin*GPC]
    for pc in percore:
        pc["pm"] = np.ascontiguousarray(
            pc["pm"].reshape(P, nwin * GPC).astype(BF16))

    meta = dict(nwin=int(nwin), cpw=tuple(int(v) for v in cpw),
                off=tuple(int(v) for v in off), totch=totch,
                seqc=tuple(int(v) for v in seqc),
                hoff=tuple(int(v) for v in hoff),
                hcols=max(hrows // P, 1) * D,
                hdupc=tuple(int(v) for v in hdupc),
                rdupc=tuple(int(v) for v in rdupc),
                nrows=tuple(int(v) for v in nrows))
    return meta, percore


def _reshape_w(W):
    # [D, D] -> [128, NJ*D]  with [p, j*D + o] = W[j*128 + p, o]
    W = np.asarray(W, np.float32)
    return np.ascontiguousarray(
        W.reshape(NJ, P, D).transpose(1, 0, 2).reshape(P, NJ * D).astype(BF16)
    )


def _reshape_fc(Wfc):
    # [D, OUT_D] -> [128, NJ*OUT_D] fp32
    Wfc = np.asarray(Wfc, np.float32)
    return np.ascontiguousarray(
        Wfc.reshape(NJ, P, OUT_D).transpose(1, 0, 2).reshape(P, NJ * OUT_D)
    )


_PROGRAM_CACHE = {}


def _build_program(meta0, meta1, has_bias, has_fcb, has_finb):
    stage = int(os.environ.get("K_STAGE", "0")) or 99
    f32 = mybir.dt.float32
    bf16 = mybir.dt.bfloat16
    Alu = mybir.AluOpType

    metas = (meta0, meta1)
    nc = bacc.Bacc("TRN2", num_devices=NCORES, debug=False)

    cwmax = max(max(meta0["cpw"]), max(meta1["cpw"]))

    # DRAM tensors
    xs_d, xh_d, dl_d, d2_d, idx_d, dinv_d, pm_d, ci_d, W_d, bias_d, fc_d, fcb_d = (
        [], [], [], [], [], [], [], [], [], [], [], [])
    for b in (0, 1):
        m = metas[b]
        xs_d.append(nc.dram_tensor(f"xs{b}", [N, D], bf16, kind="ExternalInput"))
        xh_d.append(nc.dram_tensor(f"xh{b}", [P, m["hcols"]], bf16,
                                   kind="ExternalInput"))
        dl_d.append(nc.dram_tensor(f"dl{b}", [P, m["totch"]], bf16,
                                   kind="ExternalInput"))
        d2_d.append(nc.dram_tensor(f"d2{b}", [P, m["totch"]], bf16,
                                   kind="ExternalInput"))
        idx_d.append(nc.dram_tensor(f"idx{b}", [P, m["totch"] * 8], mybir.dt.int16,
                                    kind="ExternalInput"))
        dinv_d.append(nc.dram_tensor(f"dinv{b}", [P, m["nwin"]], f32,
                                     kind="ExternalInput"))
        pm_d.append(nc.dram_tensor(f"pm{b}", [P, m["nwin"] * GPC], bf16,
                                   kind="ExternalInput"))
        ci_d.append(nc.dram_tensor(f"ci{b}", [P, NJ * GPC], f32,
                                   kind="ExternalInput"))
        W_d.append(nc.dram_tensor(f"W{b}", [P, NJ * D], bf16, kind="ExternalInput"))
        fc_d.append(nc.dram_tensor(f"fc{b}", [P, NJ * OUT_D], f32,
                                   kind="ExternalInput"))
        if has_bias[b]:
            bias_d.append(nc.dram_tensor(f"bias{b}", [1, D], bf16,
                                         kind="ExternalInput"))
        else:
            bias_d.append(None)
        if has_fcb[b]:
            fcb_d.append(nc.dram_tensor(f"fcb{b}", [1, OUT_D], f32,
                                        kind="ExternalInput"))
        else:
            fcb_d.append(None)
    fin_d = nc.dram_tensor("fin", [P, 2], f32, kind="ExternalInput")
    iota_d = nc.dram_tensor("iotam", [P, P], bf16, kind="ExternalInput")
    finb_d = nc.dram_tensor("finb", [1, 1], f32, kind="ExternalInput") if has_finb else None
    out_d = nc.dram_tensor("out", [GPC, 1], f32, kind="ExternalOutput")

    if os.environ.get("K_TRIVIAL", "") == "1":
        with tile.TileContext(nc) as tc:
            with tc.tile_pool(name="triv", bufs=1) as pool:
                t = pool.tile([GPC, 1], f32, tag="t", name="t")
                nc.sync.dma_start(out=t[:], in_=fin_d.ap()[0:GPC, 0:1])
                nc.sync.dma_start(out=out_d.ap(), in_=t[:])
        nc.compile()
        return nc

    with tile.TileContext(nc) as tc:
        with (
            tc.tile_pool(name="const", bufs=1) as cpool,
            tc.tile_pool(name="xg", bufs=3) as xgpool,
            tc.tile_pool(name="ohp", bufs=2) as ohpool,
            tc.tile_pool(name="work", bufs=2) as wpool,
            tc.tile_pool(name="pz", bufs=1, space="PSUM") as pz,
            tc.tile_pool(name="pzt", bufs=1, space="PSUM") as pzt,
            tc.tile_pool(name="ph", bufs=1, space="PSUM") as ph,
            tc.tile_pool(name="pacc", bufs=1, space="PSUM") as pacc,
        ):
            identity = cpool.tile([P, P], f32, tag="ident", name="ident")
            make_identity(nc, identity[:])
            identity_bf = cpool.tile([P, P], bf16, tag="identbf", name="identbf")
            make_identity(nc, identity_bf[:])
            iotam = cpool.tile([P, P], bf16, tag="iotam", name="iotam")
            nc.sync.dma_start(out=iotam[:], in_=iota_d.ap())

            # resident small/medium tensors
            W_sb, fc_sb, dinv_sb, pm_sb, ci_sb, idx_sb, bias_sb, fcb_sb = (
                [], [], [], [], [], [], [], [])
            dl_sb, d2_sb = [], []
            poolacc_sb = []
            poolT_sb = []
            now = os.environ.get("K_NOW", "") == "1"
            for b in (0, 1):
                m = metas[b]
                t = cpool.tile([P, m["totch"] * 8], mybir.dt.int16, tag=f"idx{b}",
                               name=f"idx{b}sb")
                nc.sync.dma_start(out=t[:], in_=idx_d[b].ap())
                idx_sb.append(t)
                t = cpool.tile([P, m["totch"]], bf16, tag=f"dl{b}", name=f"dl{b}sb")
                nc.sync.dma_start(out=t[:], in_=dl_d[b].ap())
                dl_sb.append(t)
                t = cpool.tile([P, m["totch"]], bf16, tag=f"d2{b}", name=f"d2{b}sb")
                nc.sync.dma_start(out=t[:], in_=d2_d[b].ap())
                d2_sb.append(t)
                t = cpool.tile([P, m["nwin"]], f32, tag=f"dinv{b}", name=f"dinv{b}sb")
                nc.sync.dma_start(out=t[:], in_=dinv_d[b].ap())
                dinv_sb.append(t)
                t = cpool.tile([P, m["nwin"] * GPC], bf16, tag=f"pm{b}", name=f"pm{b}sb")
                nc.sync.dma_start(out=t[:], in_=pm_d[b].ap())
                pm_sb.append(t)
                t = cpool.tile([P, NJ * GPC], f32, tag=f"ci{b}", name=f"ci{b}sb")
                nc.sync.dma_start(out=t[:], in_=ci_d[b].ap())
                ci_sb.append(t)
                if has_bias[b]:
                    t = cpool.tile([1, D], bf16, tag=f"bias{b}", name=f"bias{b}sb")
                    nc.sync.dma_start(out=t[:], in_=bias_d[b].ap())
                    bias_sb.append(t)
                else:
                    bias_sb.append(None)
                if has_fcb[b]:
                    t = cpool.tile([1, OUT_D], f32, tag=f"fcb{b}", name=f"fcb{b}sb")
                    nc.sync.dma_start(out=t[:], in_=fcb_d[b].ap())
                    fcb_sb.append(t)
                else:
                    fcb_sb.append(None)
                poolT_sb.append(
                    cpool.tile([P, NJ * GPC], f32, tag=f"pT{b}", name=f"pT{b}sb"))
                poolacc_sb.append(
                    cpool.tile([GPC, D], f32, tag=f"pa{b}", name=f"pa{b}sb"))
            for b in (0, 1):
                t = cpool.tile([P, NJ * D], bf16, tag=f"W{b}", name=f"W{b}sb")
                if not now:
                    nc.sync.dma_start(out=t[:], in_=W_d[b].ap())
                W_sb.append(t)
                t = cpool.tile([P, NJ * OUT_D], f32, tag=f"fc{b}", name=f"fc{b}sb")
                if not now:
                    nc.sync.dma_start(out=t[:], in_=fc_d[b].ap())
                fc_sb.append(t)
            fin_sb = cpool.tile([P, 2], f32, tag="fin", name="finsb")
            nc.sync.dma_start(out=fin_sb[:], in_=fin_d.ap())
            if has_finb:
                finb_sb = cpool.tile([1, 1], f32, tag="finb", name="finbsb")
                nc.sync.dma_start(out=finb_sb[:], in_=finb_d.ap())
            if has_bias[0] or has_bias[1]:
                ones_bf = cpool.tile([1, P], bf16, tag="ones_bf", name="ones_bf")
                nc.vector.memset(ones_bf[:], 1.0)
            if has_fcb[0] or has_fcb[1] or has_finb:
                ones8 = cpool.tile([1, GPC], f32, tag="ones8", name="ones8")
                nc.vector.memset(ones8[:], 1.0)

            def do_branch(b):
                m = metas[b]
                nwin, cpw, off = m["nwin"], m["cpw"], m["off"]
                seqc, hoff, nrows = m["seqc"], m["hoff"], m["nrows"]
                hdupc, rdupc = m["hdupc"], m["rdupc"]
                maxwin = int(os.environ.get("K_MAXWIN", "0")) or nwin
                nwin_eff = min(nwin, maxwin)
                pool_ps = pacc.tile([GPC, D], f32, tag="pacc", name=f"pacc{b}")
                for w in range(nwin_eff):
                    cw, ofs = cpw[w], off[w]
                    sq = seqc[w]
                    xg = xgpool.tile([P, cwmax * D], bf16, tag="xg",
                                     name=f"xg{b}_{w}")
                    if sq > 0:
                        hcol = (hoff[w] // P) * D
                        nc.sync.dma_start(
                            out=xg[:, 0:sq * D],
                            in_=xh_d[b].ap()[:, hcol:hcol + sq * D])
                    n = nrows[w]
                    n1 = min(n, 1024)
                    n2 = n - n1
                    if n1 > 0:
                        nc.gpsimd.dma_gather(
                            out_ap=xg[:, sq * D:(sq + _cdiv(n1, P)) * D].rearrange(
                                "p (c f) -> p c f", f=D),
                            in_ap=xs_d[b].ap(),
                            idxs_ap=idx_sb[b][:, ofs * 8: ofs * 8 + n1 // 16],
                            num_idxs=n1,
                            num_idxs_reg=n1,
                            elem_size=D,
                            single_packet=False,
                        )
                    if n2 > 0:
                        nc.gpsimd.dma_gather(
                            out_ap=xg[:, (sq + 8) * D: (sq + 8 + _cdiv(n2, P)) * D]
                                .rearrange("p (c f) -> p c f", f=D),
                            in_ap=xs_d[b].ap(),
                            idxs_ap=idx_sb[b][:, ofs * 8 + 64: ofs * 8 + 64 + n2 // 16],
                            num_idxs=n2,
                            num_idxs_reg=n2,
                            elem_size=D,
                            single_packet=False,
                        )
                    oh = ohpool.tile([P, cwmax * P], bf16, tag="oh",
                                     name=f"oh{b}_{w}")
                    nc.vector.tensor_tensor(
                        out=oh[:, 0:cw * P].rearrange("p (c d) -> p c d", d=P),
                        in0=dl_sb[b][:, ofs:ofs + cw].to_broadcast([P, cw, P]),
                        in1=iotam[:].rearrange("p (c d) -> p c d", c=1)
                            .to_broadcast([P, cw, P]),
                        op=Alu.is_equal)
                    dup_spans = []
                    if hdupc[w] > 0:
                        dup_spans.append((0, hdupc[w]))
                    if rdupc[w] > 0:
                        dup_spans.append((sq, sq + rdupc[w]))
                    for si, (c0, c1) in enumerate(dup_spans):
                        dc = c1 - c0
                        oh2 = ohpool.tile([P, cwmax * P], bf16, tag="oh2",
                                          name=f"oh2{b}_{w}_{si}")
                        nc.vector.tensor_tensor(
                            out=oh2[:, 0:dc * P].rearrange("p (c d) -> p c d", d=P),
                            in0=d2_sb[b][:, ofs + c0:ofs + c1]
                                .to_broadcast([P, dc, P]),
                            in1=iotam[:].rearrange("p (c d) -> p c d", c=1)
                                .to_broadcast([P, dc, P]),
                            op=Alu.is_equal)
                        nc.vector.tensor_tensor(
                            out=oh[:, c0 * P:c1 * P], in0=oh[:, c0 * P:c1 * P],
                            in1=oh2[:, 0:dc * P], op=Alu.add)

                    z_ps = pz.tile([P, D], f32, tag="z", name=f"z{b}_{w}")
                    for c in range(cw):
                        lhsT = oh[:, c * P:(c + 1) * P]
                        st = c == 0
                        sp = c == cw - 1
                        nc.tensor.matmul(z_ps[:, 0:512], lhsT,
                                         xg[:, c * D:c * D + 512],
                                         start=st, stop=sp)
                        nc.tensor.matmul(z_ps[:, 512:1024], lhsT,
                                         xg[:, c * D + 512:(c + 1) * D],
                                         start=st, stop=sp)

                    z_sb = wpool.tile([P, D], bf16, tag="z_sb", name=f"zsb{b}_{w}")
                    nc.scalar.activation(
                        out=z_sb[:, 0:512], in_=z_ps[:, 0:512],
                        func=mybir.ActivationFunctionType.Copy,
                        scale=dinv_sb[b][:, w:w + 1])
                    nc.vector.tensor_scalar(
                        out=z_sb[:, 512:1024], in0=z_ps[:, 512:1024],
                        scalar1=dinv_sb[b][:, w:w + 1], scalar2=None,
                        op0=Alu.mult)
                    if stage < 2:
                        nc.vector.tensor_tensor(out=poolT_sb[b][:, 0:1],
                                                in0=poolT_sb[b][:, 0:1],
                                                in1=z_sb[:, 0:1], op=Alu.add)
                        continue

                    zT_ps = pzt.tile([P, D], bf16, tag="zt", name=f"zt{b}_{w}")
                    for j in range(NJ):
                        nc.tensor.transpose(
                            zT_ps[:, j * P:(j + 1) * P],
                            z_sb[:, j * P:(j + 1) * P],
                            identity_bf[:])
                    zT_sb = wpool.tile([P, D], bf16, tag="zt_sb", name=f"ztsb{b}_{w}")
                    nc.scalar.copy(out=zT_sb[:, 0:512], in_=zT_ps[:, 0:512])
                    nc.vector.tensor_copy(out=zT_sb[:, 512:1024],
                                          in_=zT_ps[:, 512:1024])
                    if stage < 3:
                        nc.vector.tensor_tensor(out=poolT_sb[b][:, 0:1],
                                                in0=poolT_sb[b][:, 0:1],
                                                in1=zT_sb[:, 0:1], op=Alu.add)
                        continue

                    h_ps = ph.tile([P, D], f32, tag="h", name=f"h{b}_{w}")
                    for j in range(NJ):
                        lhsT = zT_sb[:, j * P:(j + 1) * P]
                        st = j == 0
                        sp = (j == NJ - 1) and not has_bias[b]
                        nc.tensor.matmul(h_ps[:, 0:512], lhsT,
                                         W_sb[b][:, j * D:j * D + 512],
                                         start=st, stop=sp)
                        nc.tensor.matmul(h_ps[:, 512:1024], lhsT,
                                         W_sb[b][:, j * D + 512:(j + 1) * D],
                                         start=st, stop=sp)
                    if has_bias[b]:
                        nc.tensor.matmul(h_ps[:, 0:512], ones_bf[:],
                                         bias_sb[b][:, 0:512], start=False, stop=True)
                        nc.tensor.matmul(h_ps[:, 512:1024], ones_bf[:],
                                         bias_sb[b][:, 512:1024], start=False,
                                         stop=True)

                    y = wpool.tile([P, D], bf16, tag="y", name=f"y{b}_{w}")
                    nc.scalar.activation(
                        out=y[:], in_=h_ps[:],
                        func=mybir.ActivationFunctionType.Lrelu, alpha=0.01)
                    if stage < 4:
                        nc.vector.tensor_tensor(out=poolT_sb[b][:, 0:1],
                                                in0=poolT_sb[b][:, 0:1],
                                                in1=y[:, 0:1], op=Alu.add)
                        continue

                    plhsT = pm_sb[b][:, w * GPC:(w + 1) * GPC]
                    st = w == 0
                    sp = w == nwin_eff - 1
                    nc.tensor.matmul(pool_ps[:, 0:512], plhsT, y[:, 0:512],
                                     start=st, stop=sp, skip_group_check=True)
                    nc.tensor.matmul(pool_ps[:, 512:1024], plhsT,
                                     y[:, 512:1024], start=st, stop=sp,
                                     skip_group_check=True)

                nc.scalar.copy(out=poolacc_sb[b][:], in_=pool_ps[:])
                pt_ps = pzt.tile([P, NJ * GPC], f32, tag="ptt", name=f"pt{b}ps")
                for j in range(NJ):
                    nc.tensor.transpose(
                        pt_ps[:, j * GPC:(j + 1) * GPC],
                        poolacc_sb[b][0:GPC, j * P:(j + 1) * P],
                        identity[0:GPC, 0:GPC])
                nc.vector.tensor_tensor(out=poolT_sb[b][:],
                                        in0=pt_ps[:, 0:NJ * GPC],
                                        in1=ci_sb[b][:], op=Alu.mult)

            def head_branch(b):
                # first FC layer for branch b: y1T[b] = lrelu(fc^T @ poolT)
                h1_full = ph.tile([P, D], f32, tag="h", name=f"h1_{b}ps")
                h1_ps = h1_full[:, 0:GPC]
                for j in range(NJ):
                    nc.tensor.matmul(
                        h1_ps,
                        fc_sb[b][:, j * OUT_D:(j + 1) * OUT_D],
                        poolT_sb[b][:, j * GPC:(j + 1) * GPC],
                        start=(j == 0), stop=(j == NJ - 1) and not has_fcb[b])
                if has_fcb[b]:
                    nc.tensor.matmul(h1_ps, fcb_sb[b][:],
                                     ones8[:], start=False, stop=True)
                t = cpool.tile([P, GPC], f32, tag=f"y1T{b}", name=f"y1T{b}sb")
                nc.scalar.activation(
                    out=t[:], in_=h1_ps,
                    func=mybir.ActivationFunctionType.Lrelu, alpha=0.01)
                return t

            repeat = int(os.environ.get("K_REPEAT", "1"))
            nohead = os.environ.get("K_NOHEAD", "") == "1"
            for _rep in range(repeat):
                y1T = [None, None]
                for b in (0, 1):
                    do_branch(b)
                    if not nohead:
                        y1T[b] = head_branch(b)

                if nohead:
                    out_sb = cpool.tile([GPC, 1], f32, tag="out_sb", name="out_sb")
                    nc.vector.tensor_copy(out=out_sb[:], in_=poolT_sb[0][0:GPC, 0:1])
                    nc.sync.dma_start(out=out_d.ap(), in_=out_sb[:])
                else:
                    out_full = pz.tile([P, D], f32, tag="z", name="outps")
                    out_ps = out_full[0:GPC, 0:1]
                    nc.tensor.matmul(out_ps, y1T[0][:],
                                     fin_sb[:, 0:1], start=True, stop=False)
                    nc.tensor.matmul(out_ps, y1T[1][:],
                                     fin_sb[:, 1:2],
                                     start=False, stop=not has_finb)
                    if has_finb:
                        nc.tensor.matmul(out_ps, ones8[:],
                                         finb_sb[:], start=False, stop=True)
                    out_sb = cpool.tile([GPC, 1], f32, tag="out_sb", name="out_sb")
                    nc.vector.tensor_copy(out=out_sb[:], in_=out_ps)
                    nc.sync.dma_start(out=out_d.ap(), in_=out_sb[:])

    nc.compile()
    return nc


def _prep_all(inputs):
    """Full host prep: graph assignment, both branch metas, per-core input maps."""
    assign = _assign_graphs(inputs["pro1_batch"], inputs["pro2_batch"])
    meta0, pc0 = _prep_branch(inputs["pro1_x"], inputs["pro1_edge_index"],
                              inputs["pro1_batch"], assign,
                              fullpad_windows=(0, 1, 2))
    meta1, pc1 = _prep_branch(inputs["pro2_x"], inputs["pro2_edge_index"],
                              inputs["pro2_batch"], assign)

    Wr = (_reshape_w(inputs["W1"]), _reshape_w(inputs["W2"]))
    fcr = (_reshape_fc(inputs["fc1_W"]), _reshape_fc(inputs["fc2_W"]))
    fin = np.ascontiguousarray(
        np.asarray(inputs["final_W"], np.float32).reshape(2, P).T)

    b1 = np.asarray(inputs["b1"], np.float32)
    b2 = np.asarray(inputs["b2"], np.float32)
    fc1_b = np.asarray(inputs["fc1_b"], np.float32)
    fc2_b = np.asarray(inputs["fc2_b"], np.float32)
    final_b = np.asarray(inputs["final_b"], np.float32)
    has_bias = (bool(np.any(b1)), bool(np.any(b2)))
    has_fcb = (bool(np.any(fc1_b)), bool(np.any(fc2_b)))
    has_finb = bool(np.any(final_b))

    in_maps = []
    for c in range(NCORES):
        m = {}
        for b, pc in ((0, pc0), (1, pc1)):
            d = pc[c]
            m[f"xs{b}"] = d["xs"]
            m[f"xh{b}"] = d["xh"]
            m[f"dl{b}"] = d["dl"]
            m[f"d2{b}"] = d["d2"]
            m[f"idx{b}"] = d["idx"]
            m[f"dinv{b}"] = d["dinv"]
            m[f"pm{b}"] = d["pm"]
            m[f"ci{b}"] = d["ci"]
            m[f"W{b}"] = Wr[b]
            m[f"fc{b}"] = fcr[b]
            if has_bias[b]:
                m[f"bias{b}"] = (b1 if b == 0 else b2).reshape(1, D).astype(BF16)
            if has_fcb[b]:
                m[f"fcb{b}"] = (fc1_b if b == 0 else fc2_b).reshape(1, OUT_D)
        m["fin"] = fin
        m["iotam"] = _IOTAM
        if has_finb:
            m["finb"] = final_b.reshape(1, 1)
        in_maps.append(m)

    return assign, meta0, meta1, has_bias, has_fcb, has_finb, in_maps


def kernel(pro1_x, pro1_edge_index, pro1_batch, pro2_x, pro2_edge_index, pro2_batch,
           W1, b1, fc1_W, fc1_b, W2, b2, fc2_W, fc2_b, final_W, final_b):
    inputs = dict(pro1_x=pro1_x, pro1_edge_index=pro1_edge_index,
                  pro1_batch=pro1_batch, pro2_x=pro2_x,
                  pro2_edge_index=pro2_edge_index, pro2_batch=pro2_batch,
                  W1=W1, b1=b1, fc1_W=fc1_W, fc1_b=fc1_b,
                  W2=W2, b2=b2, fc2_W=fc2_W, fc2_b=fc2_b,
                  final_W=final_W, final_b=final_b)
    (assign, meta0, meta1, has_bias, has_fcb, has_finb,
     in_maps) = _prep_all(inputs)

    key = (meta0["nwin"], meta0["cpw"], meta0["seqc"], meta0["nrows"],
           meta0["hdupc"], meta0["rdupc"],
           meta1["nwin"], meta1["cpw"], meta1["seqc"], meta1["nrows"],
           meta1["hdupc"], meta1["rdupc"],
           has_bias, has_fcb, has_finb)
    nc = _PROGRAM_CACHE.get(key)
    if nc is None:
        nc = _build_program(meta0, meta1, has_bias, has_fcb, has_finb)
        _PROGRAM_CACHE[key] = nc

    res = bass_utils.run_bass_kernel_spmd(
        nc, in_maps, core_ids=list(range(NCORES)), trace=TRACE[0])
    LAST_RESULTS[0] = res
    out = np.zeros((B, 1), np.float32)
    for c in range(NCORES):
        oc = np.asarray(res.results[c]["out"], np.float32)
        for j, g in enumerate(assign[c]):
            out[g, 0] = oc[j, 0]
    return out


# revision 34
# speedup vs baseline: 1.5000x; 1.1701x over previous
"""Trainium2 Bass kernel for a 2-branch GCN (gcn_conv -> leaky_relu -> mean-pool -> fc
head), data-parallel over graphs across 8 NeuronCores.

Math: gcn_conv(x, A, W) = (A_norm @ (x @ W)) + b = ((A_norm @ x) @ W) + b, so we
aggregate raw node features first (gather straight from the input in bf16), then do
the dense 1024x1024 matmul on each core's node shard only. The normalization
dinv[src]*dinv[dst] factorizes: scale x rows by dinv on the host, scale aggregated
rows by dinv on the device; self-loop (1/deg) folds in as a plain self-edge.

v2 layout: graphs are assigned to cores by a joint (branch1, branch2) node-count
balance so both branches fit nwin=10 windows of 128 dst nodes. Within a core, dst
nodes are bin-packed into windows by a greedy that minimizes the window's unique
gathered-row count (a src row used by several dsts of the same window is gathered
once; its one-hot column carries multiple ones via a second "dup" dl layer). Pad
slots use negative gather indices (the DMA skips them); the first three windows
of branch 0 are fully padded with row 0 so stale SBUF is never NaN.

Edges are routed by dst; segment-sum over edges is done on the TensorEngine as
one-hot matmuls (onehot[slot, dst_pos]^T @ gathered_rows accumulated in PSUM).
Pooling contracts nodes on the PE too, producing pool already transposed
[feat, graph] so the FC head needs no further transposes. Output rows per core
are its GPC assigned graphs; the host scatters them back to graph order.
"""

import os
import sys

for _p in ("/opt/trn_rl_repo",):
    if _p not in sys.path:
        sys.path.insert(0, _p)

import numpy as np
import ml_dtypes

import concourse.bacc as bacc
import concourse.mybir as mybir
import concourse.tile as tile
from concourse import bass_utils
from concourse.masks import make_identity

N, E, B, D, OUT_D = 10000, 160000, 64, 1024, 128
NCORES = 8
GPC = B // NCORES  # graphs per core
P = 128
NJ = D // P  # feature chunks (8)

BF16 = ml_dtypes.bfloat16

TRACE = [False]  # test.py can flip this for profiled runs
LAST_RESULTS = [None]

_IOTAM = np.ascontiguousarray(
    np.tile(np.arange(128, dtype=np.float32)[None, :], (128, 1))).astype(BF16)


def _cdiv(a, b):
    return -(-a // b)


def _assign_graphs(batch1, batch2):
    """Joint LPT assignment of the 64 graphs to 8 cores (GPC graphs each),
    balancing node counts of both branches simultaneously."""
    s1 = np.bincount(np.asarray(batch1, np.int64), minlength=B)
    s2 = np.bincount(np.asarray(batch2, np.int64), minlength=B)
    order = np.argsort(-(s1 + s2), kind="stable")
    loads1 = np.zeros(NCORES)
    loads2 = np.zeros(NCORES)
    counts = np.zeros(NCORES, np.int64)
    assign = [[] for _ in range(NCORES)]
    for g in order:
        best = None
        for c in range(NCORES):
            if counts[c] >= GPC:
                continue
            key = (max(loads1[c] + s1[g], loads2[c] + s2[g]),
                   loads1[c] + loads2[c])
            if best is None or key < best[0]:
                best = (key, c)
        c = best[1]
        assign[c].append(int(g))
        loads1[c] += s1[g]
        loads2[c] += s2[g]
        counts[c] += 1
    return assign


def _prep_branch(x, edge_index, batch, assign, fullpad_windows=()):
    """Host-side index preprocessing + array layout for one GCN branch.

    Returns (meta, percore). meta = dict(nwin, cpw, off, totch, seqc, hoff,
    hrows, nrows); percore[c] = dict of arrays named without branch suffix.

    Each window's slots are [home rows | random rows | pads]. Home rows are
    rows whose first use on this core is this window; they are stored
    contiguously (slot order) in the per-core xh array and loaded with one
    sequential DMA. Remaining rows are fetched by indexed gather from xs.
    A slot may serve up to two dst positions (dl + d2 one-hot layers); rows
    with more uses occupy several slots.
    """
    x = np.asarray(x, np.float32)
    src = np.asarray(edge_index[0], np.int64)
    dst = np.asarray(edge_index[1], np.int64)
    batch = np.asarray(batch, np.int64)

    cnt = np.bincount(batch, minlength=B)
    gstart = np.zeros(B + 1, np.int64)
    gstart[1:] = np.cumsum(cnt)

    deg = np.bincount(dst, minlength=N).astype(np.float32) + 1.0
    dinv = (1.0 / np.sqrt(deg)).astype(np.float32)
    xs = (x * dinv[:, None]).astype(BF16)

    corec = np.empty(N, np.int64)
    for c, gs in enumerate(assign):
        for g in gs:
            corec[gstart[g]:gstart[g + 1]] = c

    nodes_c = [int(sum(cnt[g] for g in assign[c])) for c in range(NCORES)]
    nwin = _cdiv(max(nodes_c), P)

    # --- per-core window packing ---
    # percore_win[c] = list over windows of (dsts, slots) where slots is a list
    # of (src_row, pos1, pos2) with pos2 == -1 for single-dst slots.
    order_e = np.argsort(dst, kind="stable")
    es_all, ed_all = src[order_e], dst[order_e]
    dsplit = np.searchsorted(ed_all, np.arange(N + 1))

    percore_win = []
    counts_w = np.zeros((NCORES, nwin), np.int64)
    dups_w = np.zeros((NCORES, nwin), np.int64)
    for c in range(NCORES):
        nodes = np.concatenate(
            [np.arange(gstart[g], gstart[g + 1]) for g in assign[c]])
        # unique-src sets per dst (self included)
        items = []
        for d in nodes:
            s = es_all[dsplit[d]:dsplit[d + 1]]
            u = set(s.tolist())
            u.add(int(d))
            items.append((len(u), int(d), u, s))
        items.sort(key=lambda t: -t[0])
        rows_w = [0] * nwin
        ndst_w = [0] * nwin
        sets = [set() for _ in range(nwin)]
        wins = [[] for _ in range(nwin)]
        for ln, d, u, s in items:
            best = None
            for w in range(nwin):
                if ndst_w[w] >= P:
                    continue
                inc = len(u - sets[w])
                key = (rows_w[w] + inc, ndst_w[w])
                if best is None or key < best[0]:
                    best = (key, w, inc)
            _, w, inc = best
            sets[w] |= u
            rows_w[w] += inc
            ndst_w[w] += 1
            wins[w].append((d, s))
        # slots per window: home rows first (stable storage order), then the
        # rest. A row may be homed in up to HOMES windows (first uses); its
        # remaining uses go through the indexed gather.
        HOMES = int(os.environ.get("K_HOMES", "2"))
        win_slots = []
        homed = {}
        for w in range(nwin):
            # row -> list of dst positions (with edge multiplicity)
            rowmap = {}
            for pos, (d, s) in enumerate(wins[w]):
                for sv in s.tolist():
                    rowmap.setdefault(sv, []).append(pos)
                rowmap.setdefault(int(d), []).append(pos)
            home_slots = []
            rand_slots = []
            for r, plist in rowmap.items():
                slots_r = []
                i = 0
                while i + 1 < len(plist):
                    slots_r.append((r, plist[i], plist[i + 1]))
                    i += 2
                if i < len(plist):
                    slots_r.append((r, plist[i], -1))
                if homed.get(r, 0) < HOMES:
                    homed[r] = homed.get(r, 0) + 1
                    home_slots.append(slots_r[0])
                    rand_slots.extend(slots_r[1:])
                else:
                    rand_slots.extend(slots_r)
            # pairs first within each region -> compact dl2 (dup) chunks
            home_slots.sort(key=lambda t: t[2] < 0)
            rand_slots.sort(key=lambda t: t[2] < 0)
            counts_w[c, w] = len(home_slots) * (1 << 20) + len(rand_slots)
            win_slots.append((wins[w], home_slots, rand_slots))
        percore_win.append(win_slots)

    home_w = counts_w >> 20
    rand_w = counts_w & ((1 << 20) - 1)
    seqc = (home_w.min(axis=0) // P).astype(np.int64)
    # rows demoted from home to random when a core has more homes than seqc*128
    rand_eff = rand_w + (home_w - seqc[None, :] * P)
    randc = _cdiv(rand_eff.max(axis=0), P).astype(np.int64)
    cpw = (seqc + np.maximum(randc, 0)).astype(np.int64)
    off = np.zeros(nwin + 1, np.int64)
    off[1:] = np.cumsum(cpw)
    totch = int(off[-1])
    hoff = np.zeros(nwin + 1, np.int64)
    hoff[1:] = np.cumsum(seqc * P)
    hrows = int(hoff[-1])
    nrows = [int(_cdiv(rand_eff[:, w].max(), 16) * 16) for w in range(nwin)]
    for w in fullpad_windows:
        nrows[w] = int((cpw[w] - seqc[w]) * P)
    # dup (dl2) chunk spans: home pairs live at window chunks [0, hdupc);
    # random pairs at [seqc, seqc + rdupc). Demotion only moves singles, and
    # only when a core has more TOTAL homes than seqc*128; its pairs stay in
    # the home region iff pairs <= seqc*128 — else some pair slots land in the
    # random region start, still inside [seqc, seqc+rdupc) after its singles
    # sort. Track pair counts per region exactly below instead of guessing.
    hpair_w = np.zeros((NCORES, nwin), np.int64)
    rpair_w = np.zeros((NCORES, nwin), np.int64)
    for c in range(NCORES):
        for w in range(nwin):
            _, home_slots, rand_slots = percore_win[c][w]
            nseq = int(seqc[w]) * P
            kept = home_slots[:nseq]
            demoted = home_slots[nseq:]
            hpair_w[c, w] = sum(1 for t in kept if t[2] >= 0)
            merged = [t for t in demoted if t[2] >= 0] + \
                     [t for t in rand_slots if t[2] >= 0]
            rpair_w[c, w] = len(merged)
    hdupc = _cdiv(hpair_w.max(axis=0), P).astype(np.int64)
    rdupc = _cdiv(rpair_w.max(axis=0), P).astype(np.int64)

    percore = []
    for c in range(NCORES):
        g2local = {g: j for j, g in enumerate(assign[c])}
        dl1 = np.full((totch * P,), -1.0, np.float32)
        dl2 = np.full((totch * P,), -1.0, np.float32)
        srcv = np.full((totch * P,), -1, np.int64)
        hcol = [int(hoff[w] // P) * D for w in range(nwin)]
        xh = np.zeros((P, max(hrows // P, 1) * D), BF16)
        dinv_col = np.zeros((P, nwin), np.float32)
        pm4 = np.zeros((P, nwin, GPC), np.float32)
        for w in range(nwin):
            dsts, home_slots, rand_slots = percore_win[c][w]
            nseq = int(seqc[w]) * P
            # demote excess home slots to the random region (pairs first there
            # too, so the dl2 pass stays compact)
            kept = home_slots[:nseq]
            rest = home_slots[nseq:] + rand_slots
            rest.sort(key=lambda t: t[2] < 0)
            slots = kept + rest
            base = off[w] * P
            for i, (r, p1, p2) in enumerate(slots):
                dl1[base + i] = p1
                dl2[base + i] = p2 if p2 >= 0 else -1.0
                if i < nseq:
                    # partition-major home storage: slot i -> row i % P,
                    # column block hcol[w] + (i // P)
                    xh[i % P, hcol[w] + (i // P) * D: hcol[w] + (i // P + 1) * D] \
                        = xs[r]
                else:
                    srcv[base + i] = r
            # pad slots of fully-padded windows gather row 0 (content unused)
            if w in fullpad_windows:
                n = len(slots)
                srcv[base + n: base + cpw[w] * P] = 0
            for pos, (d, s) in enumerate(dsts):
                dinv_col[pos, w] = dinv[d]
                pm4[pos, w, g2local[batch[d]]] = 1.0

        dl1m = np.ascontiguousarray(
            dl1.reshape(totch, P).T).astype(BF16)  # [128, totch]
        dl2m = np.ascontiguousarray(
            dl2.reshape(totch, P).T).astype(BF16)  # [128, totch]

        # int16 gather indices for the random region, wrapped per call:
        # within a call, row e -> [e % 16, base + e // 16], replicated to 128.
        idx = np.full((16, totch * 8), -1, np.int16)
        for w in range(nwin):
            rbase = off[w] * P + int(seqc[w]) * P
            n = nrows[w]
            n1 = min(n, 1024)
            n2 = n - n1
            sw = srcv[rbase: rbase + n1]
            idx[:, off[w] * 8: off[w] * 8 + n1 // 16] = (
                sw.astype(np.int16).reshape(-1, 16).T)
            if n2 > 0:
                sw = srcv[rbase + 1024: rbase + 1024 + n2]
                idx[:, off[w] * 8 + 64: off[w] * 8 + 64 + n2 // 16] = (
                    sw.astype(np.int16).reshape(-1, 16).T)
        idx = np.ascontiguousarray(np.tile(idx, (8, 1)))

        civ = (1.0 / np.maximum(
            [cnt[g] for g in assign[c]], 1)).astype(np.float32)
        ci = np.ascontiguousarray(np.tile(civ, (P, NJ)))  # [128, NJ*GPC]

        percore.append(
            dict(xs=xs, xh=np.ascontiguousarray(xh), dl=dl1m, d2=dl2m, idx=idx,
                 dinv=dinv_col, pm=pm4, ci=ci))

    # pm flatten: [128, nwin*GPC]
    for pc in percore:
        pc["pm"] = np.ascontiguousarray(
            pc["pm"].reshape(P, nwin * GPC).astype(BF16))

    meta = dict(nwin=int(nwin), cpw=tuple(int(v) for v in cpw),
                off=tuple(int(v) for v in off), totch=totch,
                seqc=tuple(int(v) for v in seqc),
                hoff=tuple(int(v) for v in hoff),
                hcols=max(hrows // P, 1) * D,
                hdupc=tuple(int(v) for v in hdupc),
                rdupc=tuple(int(v) for v in rdupc),
                nrows=tuple(int(v) for v in nrows))
    return meta, percore


def _reshape_w(W):
    # [D, D] -> [128, NJ*D]  with [p, j*D + o] = W[j*128 + p, o]
    W = np.asarray(W, np.float32)
    return np.ascontiguousarray(
        W.reshape(NJ, P, D).transpose(1, 0, 2).reshape(P, NJ * D).astype(BF16)
    )


def _reshape_fc(Wfc):
    # [D, OUT_D] -> [128, NJ*OUT_D] fp32
    Wfc = np.asarray(Wfc, np.float32)
    return np.ascontiguousarray(
        Wfc.reshape(NJ, P, OUT_D).transpose(1, 0, 2).reshape(P, NJ * OUT_D)
    )


_PROGRAM_CACHE = {}


def _build_program(meta0, meta1, has_bias, has_fcb, has_finb):
    stage = int(os.environ.get("K_STAGE", "0")) or 99
    f32 = mybir.dt.float32
    bf16 = mybir.dt.bfloat16
    Alu = mybir.AluOpType

    metas = (meta0, meta1)
    nc = bacc.Bacc("TRN2", num_devices=NCORES, debug=False)

    cwmax = max(max(meta0["cpw"]), max(meta1["cpw"]))

    # DRAM tensors
    xs_d, xh_d, dl_d, d2_d, idx_d, dinv_d, pm_d, ci_d, W_d, bias_d, fc_d, fcb_d = (
        [], [], [], [], [], [], [], [], [], [], [], [])
    for b in (0, 1):
        m = metas[b]
        xs_d.append(nc.dram_tensor(f"xs{b}", [N, D], bf16, kind="ExternalInput"))
        xh_d.append(nc.dram_tensor(f"xh{b}", [P, m["hcols"]], bf16,
                                   kind="ExternalInput"))
        dl_d.append(nc.dram_tensor(f"dl{b}", [P, m["totch"]], bf16,
                                   kind="ExternalInput"))
        d2_d.append(nc.dram_tensor(f"d2{b}", [P, m["totch"]], bf16,
                                   kind="ExternalInput"))
        idx_d.append(nc.dram_tensor(f"idx{b}", [P, m["totch"] * 8], mybir.dt.int16,
                                    kind="ExternalInput"))
        dinv_d.append(nc.dram_tensor(f"dinv{b}", [P, m["nwin"]], f32,
                                     kind="ExternalInput"))
        pm_d.append(nc.dram_tensor(f"pm{b}", [P, m["nwin"] * GPC], bf16,
                                   kind="ExternalInput"))
        ci_d.append(nc.dram_tensor(f"ci{b}", [P, NJ * GPC], f32,
                                   kind="ExternalInput"))
        W_d.append(nc.dram_tensor(f"W{b}", [P, NJ * D], bf16, kind="ExternalInput"))
        fc_d.append(nc.dram_tensor(f"fc{b}", [P, NJ * OUT_D], f32,
                                   kind="ExternalInput"))
        if has_bias[b]:
            bias_d.append(nc.dram_tensor(f"bias{b}", [1, D], bf16,
                                         kind="ExternalInput"))
        else:
            bias_d.append(None)
        if has_fcb[b]:
            fcb_d.append(nc.dram_tensor(f"fcb{b}", [1, OUT_D], f32,
                                        kind="ExternalInput"))
        else:
            fcb_d.append(None)
    fin_d = nc.dram_tensor("fin", [P, 2], f32, kind="ExternalInput")
    iota_d = nc.dram_tensor("iotam", [P, P], bf16, kind="ExternalInput")
    finb_d = nc.dram_tensor("finb", [1, 1], f32, kind="ExternalInput") if has_finb else None
    out_d = nc.dram_tensor("out", [GPC, 1], f32, kind="ExternalOutput")

    if os.environ.get("K_TRIVIAL", "") == "1":
        with tile.TileContext(nc) as tc:
            with tc.tile_pool(name="triv", bufs=1) as pool:
                t = pool.tile([GPC, 1], f32, tag="t", name="t")
                nc.sync.dma_start(out=t[:], in_=fin_d.ap()[0:GPC, 0:1])
                nc.sync.dma_start(out=out_d.ap(), in_=t[:])
        nc.compile()
        return nc

    with tile.TileContext(nc) as tc:
        with (
            tc.tile_pool(name="const", bufs=1) as cpool,
            tc.tile_pool(name="xg", bufs=3) as xgpool,
            tc.tile_pool(name="ohp", bufs=2) as ohpool,
            tc.tile_pool(name="work", bufs=2) as wpool,
            tc.tile_pool(name="pz", bufs=1, space="PSUM") as pz,
            tc.tile_pool(name="pzt", bufs=1, space="PSUM") as pzt,
            tc.tile_pool(name="ph", bufs=1, space="PSUM") as ph,
            tc.tile_pool(name="pacc", bufs=1, space="PSUM") as pacc,
        ):
            identity = cpool.tile([P, P], f32, tag="ident", name="ident")
            make_identity(nc, identity[:])
            identity_bf = cpool.tile([P, P], bf16, tag="identbf", name="identbf")
            make_identity(nc, identity_bf[:])
            iotam = cpool.tile([P, P], bf16, tag="iotam", name="iotam")
            nc.sync.dma_start(out=iotam[:], in_=iota_d.ap())

            # resident small/medium tensors
            W_sb, fc_sb, dinv_sb, pm_sb, ci_sb, idx_sb, bias_sb, fcb_sb = (
                [], [], [], [], [], [], [], [])
            dl_sb, d2_sb = [], []
            poolacc_sb = []
            poolT_sb = []
            now = os.environ.get("K_NOW", "") == "1"
            for b in (0, 1):
                m = metas[b]
                t = cpool.tile([P, m["totch"] * 8], mybir.dt.int16, tag=f"idx{b}",
                               name=f"idx{b}sb")
                nc.sync.dma_start(out=t[:], in_=idx_d[b].ap())
                idx_sb.append(t)
                t = cpool.tile([P, m["totch"]], bf16, tag=f"dl{b}", name=f"dl{b}sb")
                nc.sync.dma_start(out=t[:], in_=dl_d[b].ap())
                dl_sb.append(t)
                t = cpool.tile([P, m["totch"]], bf16, tag=f"d2{b}", name=f"d2{b}sb")
                nc.sync.dma_start(out=t[:], in_=d2_d[b].ap())
                d2_sb.append(t)
                t = cpool.tile([P, m["nwin"]], f32, tag=f"dinv{b}", name=f"dinv{b}sb")
                nc.sync.dma_start(out=t[:], in_=dinv_d[b].ap())
                dinv_sb.append(t)
                t = cpool.tile([P, m["nwin"] * GPC], bf16, tag=f"pm{b}", name=f"pm{b}sb")
                nc.sync.dma_start(out=t[:], in_=pm_d[b].ap())
                pm_sb.append(t)
                t = cpool.tile([P, NJ * GPC], f32, tag=f"ci{b}", name=f"ci{b}sb")
                nc.sync.dma_start(out=t[:], in_=ci_d[b].ap())
                ci_sb.append(t)
                if has_bias[b]:
                    t = cpool.tile([1, D], bf16, tag=f"bias{b}", name=f"bias{b}sb")
                    nc.sync.dma_start(out=t[:], in_=bias_d[b].ap())
                    bias_sb.append(t)
                else:
                    bias_sb.append(None)
                if has_fcb[b]:
                    t = cpool.tile([1, OUT_D], f32, tag=f"fcb{b}", name=f"fcb{b}sb")
                    nc.sync.dma_start(out=t[:], in_=fcb_d[b].ap())
                    fcb_sb.append(t)
                else:
                    fcb_sb.append(None)
                poolT_sb.append(
                    cpool.tile([P, NJ * GPC], f32, tag=f"pT{b}", name=f"pT{b}sb"))
                poolacc_sb.append(
                    cpool.tile([GPC, D], f32, tag=f"pa{b}", name=f"pa{b}sb"))
            for b in (0, 1):
                t = cpool.tile([P, NJ * D], bf16, tag=f"W{b}", name=f"W{b}sb")
                if not now:
                    nc.sync.dma_start(out=t[:], in_=W_d[b].ap())
                W_sb.append(t)
                t = cpool.tile([P, NJ * OUT_D], f32, tag=f"fc{b}", name=f"fc{b}sb")
                if not now:
                    nc.sync.dma_start(out=t[:], in_=fc_d[b].ap())
                fc_sb.append(t)
            fin_sb = cpool.tile([P, 2], f32, tag="fin", name="finsb")
            nc.sync.dma_start(out=fin_sb[:], in_=fin_d.ap())
            if has_finb:
                finb_sb = cpool.tile([1, 1], f32, tag="finb", name="finbsb")
                nc.sync.dma_start(out=finb_sb[:], in_=finb_d.ap())
            if has_bias[0] or has_bias[1]:
                ones_bf = cpool.tile([1, P], bf16, tag="ones_bf", name="ones_bf")
                nc.vector.memset(ones_bf[:], 1.0)
            if has_fcb[0] or has_fcb[1] or has_finb:
                ones8 = cpool.tile([1, GPC], f32, tag="ones8", name="ones8")
                nc.vector.memset(ones8[:], 1.0)

            def stage_agg(b, w):
                m = metas[b]
                cpw, off = m["cpw"], m["off"]
                seqc, hoff, nrows = m["seqc"], m["hoff"], m["nrows"]
                hdupc, rdupc = m["hdupc"], m["rdupc"]
                if True:
                    cw, ofs = cpw[w], off[w]
                    sq = seqc[w]
                    xg = xgpool.tile([P, cwmax * D], bf16, tag="xg",
                                     name=f"xg{b}_{w}")
                    if sq > 0:
                        hcol = (hoff[w] // P) * D
                        nc.sync.dma_start(
                            out=xg[:, 0:sq * D],
                            in_=xh_d[b].ap()[:, hcol:hcol + sq * D])
                    n = nrows[w]
                    n1 = min(n, 1024)
                    n2 = n - n1
                    if n1 > 0:
                        nc.gpsimd.dma_gather(
                            out_ap=xg[:, sq * D:(sq + _cdiv(n1, P)) * D].rearrange(
                                "p (c f) -> p c f", f=D),
                            in_ap=xs_d[b].ap(),
                            idxs_ap=idx_sb[b][:, ofs * 8: ofs * 8 + n1 // 16],
                            num_idxs=n1,
                            num_idxs_reg=n1,
                            elem_size=D,
                            single_packet=False,
                        )
                    if n2 > 0:
                        nc.gpsimd.dma_gather(
                            out_ap=xg[:, (sq + 8) * D: (sq + 8 + _cdiv(n2, P)) * D]
                                .rearrange("p (c f) -> p c f", f=D),
                            in_ap=xs_d[b].ap(),
                            idxs_ap=idx_sb[b][:, ofs * 8 + 64: ofs * 8 + 64 + n2 // 16],
                            num_idxs=n2,
                            num_idxs_reg=n2,
                            elem_size=D,
                            single_packet=False,
                        )
                    oh = ohpool.tile([P, cwmax * P], bf16, tag="oh",
                                     name=f"oh{b}_{w}")
                    nc.vector.tensor_tensor(
                        out=oh[:, 0:cw * P].rearrange("p (c d) -> p c d", d=P),
                        in0=dl_sb[b][:, ofs:ofs + cw].to_broadcast([P, cw, P]),
                        in1=iotam[:].rearrange("p (c d) -> p c d", c=1)
                            .to_broadcast([P, cw, P]),
                        op=Alu.is_equal)
                    dup_spans = []
                    if hdupc[w] > 0:
                        dup_spans.append((0, hdupc[w]))
                    if rdupc[w] > 0:
                        dup_spans.append((sq, sq + rdupc[w]))
                    for si, (c0, c1) in enumerate(dup_spans):
                        dc = c1 - c0
                        oh2 = ohpool.tile([P, cwmax * P], bf16, tag="oh2",
                                          name=f"oh2{b}_{w}_{si}")
                        nc.vector.tensor_tensor(
                            out=oh2[:, 0:dc * P].rearrange("p (c d) -> p c d", d=P),
                            in0=d2_sb[b][:, ofs + c0:ofs + c1]
                                .to_broadcast([P, dc, P]),
                            in1=iotam[:].rearrange("p (c d) -> p c d", c=1)
                                .to_broadcast([P, dc, P]),
                            op=Alu.is_equal)
                        nc.vector.tensor_tensor(
                            out=oh[:, c0 * P:c1 * P], in0=oh[:, c0 * P:c1 * P],
                            in1=oh2[:, 0:dc * P], op=Alu.add)

                    z_ps = pz.tile([P, D], f32, tag="z", name=f"z{b}_{w}")
                    for c in range(cw):
                        lhsT = oh[:, c * P:(c + 1) * P]
                        st = c == 0
                        sp = c == cw - 1
                        nc.tensor.matmul(z_ps[:, 0:512], lhsT,
                                         xg[:, c * D:c * D + 512],
                                         start=st, stop=sp)
                        nc.tensor.matmul(z_ps[:, 512:1024], lhsT,
                                         xg[:, c * D + 512:(c + 1) * D],
                                         start=st, stop=sp)

                    z_sb = wpool.tile([P, D], bf16, tag="z_sb", name=f"zsb{b}_{w}")
                    nc.scalar.activation(
                        out=z_sb[:, 0:512], in_=z_ps[:, 0:512],
                        func=mybir.ActivationFunctionType.Copy,
                        scale=dinv_sb[b][:, w:w + 1])
                    nc.vector.tensor_scalar(
                        out=z_sb[:, 512:1024], in0=z_ps[:, 512:1024],
                        scalar1=dinv_sb[b][:, w:w + 1], scalar2=None,
                        op0=Alu.mult)
                    if stage < 2:
                        nc.vector.tensor_tensor(out=poolT_sb[b][:, 0:1],
                                                in0=poolT_sb[b][:, 0:1],
                                                in1=z_sb[:, 0:1], op=Alu.add)
                    return z_sb

            def stage_T(b, w, z_sb):
                zT_ps = pzt.tile([P, D], bf16, tag="zt", name=f"zt{b}_{w}")
                for j in range(NJ):
                    nc.tensor.transpose(
                        zT_ps[:, j * P:(j + 1) * P],
                        z_sb[:, j * P:(j + 1) * P],
                        identity_bf[:])
                zT_sb = wpool.tile([P, D], bf16, tag="zt_sb", name=f"ztsb{b}_{w}")
                nc.scalar.copy(out=zT_sb[:, 0:512], in_=zT_ps[:, 0:512])
                nc.vector.tensor_copy(out=zT_sb[:, 512:1024],
                                      in_=zT_ps[:, 512:1024])
                if stage < 3:
                    nc.vector.tensor_tensor(out=poolT_sb[b][:, 0:1],
                                            in0=poolT_sb[b][:, 0:1],
                                            in1=zT_sb[:, 0:1], op=Alu.add)
                return zT_sb

            def stage_W(b, w, zT_sb, pool_ps, nwin_eff):
                h_ps = ph.tile([P, D], f32, tag="h", name=f"h{b}_{w}")
                for j in range(NJ):
                    lhsT = zT_sb[:, j * P:(j + 1) * P]
                    st = j == 0
                    sp = (j == NJ - 1) and not has_bias[b]
                    nc.tensor.matmul(h_ps[:, 0:512], lhsT,
                                     W_sb[b][:, j * D:j * D + 512],
                                     start=st, stop=sp)
                    nc.tensor.matmul(h_ps[:, 512:1024], lhsT,
                                     W_sb[b][:, j * D + 512:(j + 1) * D],
                                     start=st, stop=sp)
                if has_bias[b]:
                    nc.tensor.matmul(h_ps[:, 0:512], ones_bf[:],
                                     bias_sb[b][:, 0:512], start=False, stop=True)
                    nc.tensor.matmul(h_ps[:, 512:1024], ones_bf[:],
                                     bias_sb[b][:, 512:1024], start=False,
                                     stop=True)

                y = wpool.tile([P, D], bf16, tag="y", name=f"y{b}_{w}")
                nc.scalar.activation(
                    out=y[:], in_=h_ps[:],
                    func=mybir.ActivationFunctionType.Lrelu, alpha=0.01)
                if stage < 4:
                    nc.vector.tensor_tensor(out=poolT_sb[b][:, 0:1],
                                            in0=poolT_sb[b][:, 0:1],
                                            in1=y[:, 0:1], op=Alu.add)
                    return

                plhsT = pm_sb[b][:, w * GPC:(w + 1) * GPC]
                st = w == 0
                sp = w == nwin_eff - 1
                nc.tensor.matmul(pool_ps[:, 0:512], plhsT, y[:, 0:512],
                                 start=st, stop=sp, skip_group_check=True)
                nc.tensor.matmul(pool_ps[:, 512:1024], plhsT,
                                 y[:, 512:1024], start=st, stop=sp,
                                 skip_group_check=True)

            def do_branch(b):
                m = metas[b]
                nwin = m["nwin"]
                maxwin = int(os.environ.get("K_MAXWIN", "0")) or nwin
                nwin_eff = min(nwin, maxwin)
                pool_ps = (pacc.tile([GPC, D], f32, tag="pacc", name=f"pacc{b}")
                           if stage >= 4 else None)
                # depth-2 software pipeline: T lags agg by 1 window, W by 2 —
                # PE never waits on the ACT/DVE PSUM->SBUF copies.
                zq, ztq = {}, {}
                for w in range(nwin_eff):
                    zq[w] = stage_agg(b, w)
                    if stage >= 2 and w >= 1:
                        ztq[w - 1] = stage_T(b, w - 1, zq.pop(w - 1))
                    if stage >= 3 and w >= 2:
                        stage_W(b, w - 2, ztq.pop(w - 2), pool_ps, nwin_eff)
                if stage >= 2 and nwin_eff >= 1:
                    w = nwin_eff - 1
                    ztq[w] = stage_T(b, w, zq.pop(w))
                if stage >= 3:
                    for w in (nwin_eff - 2, nwin_eff - 1):
                        if w >= 0 and w in ztq:
                            stage_W(b, w, ztq.pop(w), pool_ps, nwin_eff)

                if stage < 4:
                    return
                nc.scalar.copy(out=poolacc_sb[b][:], in_=pool_ps[:])
                pt_ps = pzt.tile([P, NJ * GPC], f32, tag="ptt", name=f"pt{b}ps")
                for j in range(NJ):
                    nc.tensor.transpose(
                        pt_ps[:, j * GPC:(j + 1) * GPC],
                        poolacc_sb[b][0:GPC, j * P:(j + 1) * P],
                        identity[0:GPC, 0:GPC])
                nc.vector.tensor_tensor(out=poolT_sb[b][:],
                                        in0=pt_ps[:, 0:NJ * GPC],
                                        in1=ci_sb[b][:], op=Alu.mult)

            def head_branch(b):
                # first FC layer for branch b: y1T[b] = lrelu(fc^T @ poolT)
                h1_full = ph.tile([P, D], f32, tag="h", name=f"h1_{b}ps")
                h1_ps = h1_full[:, 0:GPC]
                for j in range(NJ):
                    nc.tensor.matmul(
                        h1_ps,
                        fc_sb[b][:, j * OUT_D:(j + 1) * OUT_D],
                        poolT_sb[b][:, j * GPC:(j + 1) * GPC],
                        start=(j == 0), stop=(j == NJ - 1) and not has_fcb[b])
                if has_fcb[b]:
                    nc.tensor.matmul(h1_ps, fcb_sb[b][:],
                                     ones8[:], start=False, stop=True)
                t = cpool.tile([P, GPC], f32, tag=f"y1T{b}", name=f"y1T{b}sb")
                nc.scalar.activation(
                    out=t[:], in_=h1_ps,
                    func=mybir.ActivationFunctionType.Lrelu, alpha=0.01)
                return t

            repeat = int(os.environ.get("K_REPEAT", "1"))
            nohead = os.environ.get("K_NOHEAD", "") == "1" or stage < 4
            for _rep in range(repeat):
                y1T = [None, None]
                for b in (0, 1):
                    do_branch(b)
                    if not nohead:
                        y1T[b] = head_branch(b)

                if nohead:
                    out_sb = cpool.tile([GPC, 1], f32, tag="out_sb", name="out_sb")
                    nc.vector.tensor_copy(out=out_sb[:], in_=poolT_sb[0][0:GPC, 0:1])
                    nc.sync.dma_start(out=out_d.ap(), in_=out_sb[:])
                else:
                    out_full = pz.tile([P, D], f32, tag="z", name="outps")
                    out_ps = out_full[0:GPC, 0:1]
                    nc.tensor.matmul(out_ps, y1T[0][:],
                                     fin_sb[:, 0:1], start=True, stop=False)
                    nc.tensor.matmul(out_ps, y1T[1][:],
                                     fin_sb[:, 1:2],
                                     start=False, stop=not has_finb)
                    if has_finb:
                        nc.tensor.matmul(out_ps, ones8[:],
                                         finb_sb[:], start=False, stop=True)
                    out_sb = cpool.tile([GPC, 1], f32, tag="out_sb", name="out_sb")
                    nc.vector.tensor_copy(out=out_sb[:], in_=out_ps)
                    nc.sync.dma_start(out=out_d.ap(), in_=out_sb[:])

    nc.compile()
    return nc


def _prep_all(inputs):
    """Full host prep: graph assignment, both branch metas, per-core input maps."""
    assign = _assign_graphs(inputs["pro1_batch"], inputs["pro2_batch"])
    meta0, pc0 = _prep_branch(inputs["pro1_x"], inputs["pro1_edge_index"],
                              inputs["pro1_batch"], assign,
                              fullpad_windows=(0, 1, 2))
    meta1, pc1 = _prep_branch(inputs["pro2_x"], inputs["pro2_edge_index"],
                              inputs["pro2_batch"], assign)

    Wr = (_reshape_w(inputs["W1"]), _reshape_w(inputs["W2"]))
    fcr = (_reshape_fc(inputs["fc1_W"]), _reshape_fc(inputs["fc2_W"]))
    fin = np.ascontiguousarray(
        np.asarray(inputs["final_W"], np.float32).reshape(2, P).T)

    b1 = np.asarray(inputs["b1"], np.float32)
    b2 = np.asarray(inputs["b2"], np.float32)
    fc1_b = np.asarray(inputs["fc1_b"], np.float32)
    fc2_b = np.asarray(inputs["fc2_b"], np.float32)
    final_b = np.asarray(inputs["final_b"], np.float32)
    has_bias = (bool(np.any(b1)), bool(np.any(b2)))
    has_fcb = (bool(np.any(fc1_b)), bool(np.any(fc2_b)))
    has_finb = bool(np.any(final_b))

    in_maps = []
    for c in range(NCORES):
        m = {}
        for b, pc in ((0, pc0), (1, pc1)):
            d = pc[c]
            m[f"xs{b}"] = d["xs"]
            m[f"xh{b}"] = d["xh"]
            m[f"dl{b}"] = d["dl"]
            m[f"d2{b}"] = d["d2"]
            m[f"idx{b}"] = d["idx"]
            m[f"dinv{b}"] = d["dinv"]
            m[f"pm{b}"] = d["pm"]
            m[f"ci{b}"] = d["ci"]
            m[f"W{b}"] = Wr[b]
            m[f"fc{b}"] = fcr[b]
            if has_bias[b]:
                m[f"bias{b}"] = (b1 if b == 0 else b2).reshape(1, D).astype(BF16)
            if has_fcb[b]:
                m[f"fcb{b}"] = (fc1_b if b == 0 else fc2_b).reshape(1, OUT_D)
        m["fin"] = fin
        m["iotam"] = _IOTAM
        if has_finb:
            m["finb"] = final_b.reshape(1, 1)
        in_maps.append(m)

    return assign, meta0, meta1, has_bias, has_fcb, has_finb, in_maps


def kernel(pro1_x, pro1_edge_index, pro1_batch, pro2_x, pro2_edge_index, pro2_batch,
           W1, b1, fc1_W, fc1_b, W2, b2, fc2_W, fc2_b, final_W, final_b):
    inputs = dict(pro1_x=pro1_x, pro1_edge_index=pro1_edge_index,
                  pro1_batch=pro1_batch, pro2_x=pro2_x,
                  pro2_edge_index=pro2_edge_index, pro2_batch=pro2_batch,
                  W1=W1, b1=b1, fc1_W=fc1_W, fc1_b=fc1_b,
                  W2=W2, b2=b2, fc2_W=fc2_W, fc2_b=fc2_b,
                  final_W=final_W, final_b=final_b)
    (assign, meta0, meta1, has_bias, has_fcb, has_finb,
     in_maps) = _prep_all(inputs)

    key = (meta0["nwin"], meta0["cpw"], meta0["seqc"], meta0["nrows"],
           meta0["hdupc"], meta0["rdupc"],
           meta1["nwin"], meta1["cpw"], meta1["seqc"], meta1["nrows"],
           meta1["hdupc"], meta1["rdupc"],
           has_bias, has_fcb, has_finb)
    nc = _PROGRAM_CACHE.get(key)
    if nc is None:
        nc = _build_program(meta0, meta1, has_bias, has_fcb, has_finb)
        _PROGRAM_CACHE[key] = nc

    res = bass_utils.run_bass_kernel_spmd(
        nc, in_maps, core_ids=list(range(NCORES)), trace=TRACE[0])
    LAST_RESULTS[0] = res
    out = np.zeros((B, 1), np.float32)
    for c in range(NCORES):
        oc = np.asarray(res.results[c]["out"], np.float32)
        for j, g in enumerate(assign[c]):
            out[g, 0] = oc[j, 0]
    return out
